# revision 7
# baseline (speedup 1.0000x reference)
"""Trainium2 Bass kernel for nn_Attention (LayerNorm + L2-normalized-QK attention
with null-kv slot + output projection), SPMD across 8 NeuronCores.

Sharding: core c = (batch b = c//2, query-half hi = c%2). Each core computes the
full kv (2048 tokens) of its batch and attention outputs for its 1024-query
half. Softmax over kv is permutation invariant, so for hi=1 we feed x with the
two sequence halves swapped — every core then runs the identical SPMD program
with its queries in rows 0:1024. The final output is a pure concatenation of
the per-core results (no collectives, no host arithmetic).

Device-side layout choices:
  - S is computed transposed (S^T [kv, q]) so no softmax row-max pass is
    needed: q,k are L2-normalized so |8*q.k| <= 8 and exp() cannot overflow.
  - PV uses V' = [V | 1] (M=65) so the softmax denominator falls out of the
    same matmul chain, and the output lands directly in A^T layout for the
    output projection.
  - rsqrt is computed as exp(-0.5*ln(x)) so the only ACT table set used is
    natural_log_exp_and_others (no table thrashing with the softmax exp).
  - all matmuls in bf16 with fp32 PSUM accumulation.
"""

import numpy as np

B = 4
N = 2048
DIM = 1024
HEADS = 16
DH = 64
INNER = HEADS * DH
NQ = 1024  # queries per core
SCALE = 8.0
LN_EPS = 1e-5

_CACHE = {}


def _build_program():
    from contextlib import ExitStack

    import concourse.bacc as bacc
    import concourse.bass as bass
    import concourse.tile as tile
    from concourse import mybir

    f32 = mybir.dt.float32
    bf16 = mybir.dt.bfloat16
    AF = mybir.ActivationFunctionType
    OP = mybir.AluOpType
    AX = mybir.AxisListType

    NT = N // 128          # 16 token tiles
    NTQ = NQ // 128        # 8 query token tiles
    NCD = DIM // 128       # 8 dim chunks
    HP = HEADS // 2        # 8 head pairs

    nc = bacc.Bacc("TRN2", target_bir_lowering=False, debug=False)

    x = nc.declare_dram_parameter("x", [N, DIM], f32, isOutput=False)
    gamma = nc.declare_dram_parameter("gamma", [DIM], f32, isOutput=False)
    beta = nc.declare_dram_parameter("beta", [DIM], f32, isOutput=False)
    Wq = nc.declare_dram_parameter("Wq", [DIM, INNER], f32, isOutput=False)
    Wk = nc.declare_dram_parameter("Wk", [DIM, INNER], f32, isOutput=False)
    Wv = nc.declare_dram_parameter("Wv", [DIM, INNER], f32, isOutput=False)
    Wo = nc.declare_dram_parameter("Wo", [INNER, DIM], f32, isOutput=False)
    nk = nc.declare_dram_parameter("nk", [HEADS, DH], f32, isOutput=False)
    nv = nc.declare_dram_parameter("nv", [HEADS, DH], f32, isOutput=False)
    qs = nc.declare_dram_parameter("qs", [DH], f32, isOutput=False)
    ks = nc.declare_dram_parameter("ks", [DH], f32, isOutput=False)
    out = nc.declare_dram_parameter("out", [NQ, DIM], f32, isOutput=True)

    # internal DRAM for transpose round-trips
    xn_d = nc.dram_tensor("xn_d", [N, DIM], bf16)
    kn_d = nc.dram_tensor("kn_d", [N, INNER], bf16)
    qn_d = nc.dram_tensor("qn_d", [NQ, INNER], bf16)
    nkn_d = nc.dram_tensor("nkn_d", [HEADS, DH], bf16)
    nvb_d = nc.dram_tensor("nvb_d", [HEADS, DH + 1], bf16)

    with tile.TileContext(nc) as tc, ExitStack() as ctx:
        singles = ctx.enter_context(tc.tile_pool(name="singles", bufs=1))
        big = ctx.enter_context(tc.tile_pool(name="big", bufs=1))

        # ---------------- persistent SBUF tensors ----------------
        xnT = big.tile([128, NCD, N], bf16, tag="xnT")       # xn^T  [dim, tok]
        kT = big.tile([128, HP, N], bf16, tag="kT")          # k^T   [2*64, kv] per pair
        qT = big.tile([128, HP, NQ], bf16, tag="qT")         # q^T
        vsb = big.tile([128, NT, HEADS, DH + 1], bf16, tag="v")   # V'=[V|1]
        AT = big.tile([128, NCD, NQ], bf16, tag="AT")        # A^T (attn out)

        # ---------------- constants ----------------
        gamma_b = singles.tile([128, DIM], f32)
        nc.gpsimd.dma_start(out=gamma_b, in_=gamma.ap().partition_broadcast(128))
        beta_b = singles.tile([128, DIM], f32)
        nc.gpsimd.dma_start(out=beta_b, in_=beta.ap().partition_broadcast(128))
        eps_t = singles.tile([128, 1], f32)
        nc.vector.memset(eps_t, LN_EPS)
        eps30 = singles.tile([128, 1], f32)
        nc.vector.memset(eps30, 1e-30)

        qs_b = singles.tile([128, DH], f32)
        nc.gpsimd.dma_start(out=qs_b, in_=qs.ap().partition_broadcast(128))
        ks_b = singles.tile([128, DH], f32)
        nc.gpsimd.dma_start(out=ks_b, in_=ks.ap().partition_broadcast(128))
        c64 = singles.tile([128, DH], f32)
        nc.vector.tensor_tensor(out=c64, in0=qs_b, in1=ks_b, op=OP.mult)
        c8 = singles.tile([128, 8, DH], f32)   # qs*ks tiled for 8 heads (one col half)
        for g in range(8):
            nc.vector.tensor_copy(out=c8[:, g, :], in_=c64)

        nc.vector.memset(vsb[:, :, :, DH : DH + 1], 1.0)  # ones column of V'

        # null-kv prep: nkn = l2norm(nk)*qs*ks (bf16), then transpose via DRAM
        nk_t = singles.tile([HEADS, DH], f32)
        nc.sync.dma_start(out=nk_t, in_=nk[:, :])
        nksq = singles.tile([HEADS, DH], f32)
        nc.vector.tensor_tensor(out=nksq, in0=nk_t, in1=nk_t, op=OP.mult)
        nks = singles.tile([HEADS, 1], f32)
        nc.vector.tensor_reduce(out=nks, in_=nksq, axis=AX.X, op=OP.add)
        nc.scalar.activation(out=nks, in_=nks, func=AF.Ln, bias=eps30[0:HEADS, :])
        nc.scalar.activation(out=nks, in_=nks, func=AF.Exp, scale=-0.5)
        nc.vector.tensor_scalar_min(out=nks, in0=nks, scalar1=1e12)
        nkn = singles.tile([HEADS, DH], f32)
        nc.vector.tensor_scalar_mul(out=nkn, in0=nk_t, scalar1=nks)
        nknb = singles.tile([HEADS, DH], bf16)
        nc.vector.tensor_tensor(out=nknb, in0=nkn, in1=c64[0:HEADS, :], op=OP.mult)
        nc.sync.dma_start(out=nkn_d[:, :], in_=nknb)
        nknT = singles.tile([DH, HEADS], bf16)
        nc.sync.dma_start(out=nknT, in_=nkn_d.ap().rearrange("h d -> d h"))
        # block-diagonal [128, 16]: col 2p rows 0:64 = head 2p, col 2p+1 rows 64:128 = head 2p+1
        nkn_bd = singles.tile([128, HEADS], bf16)
        nc.vector.memset(nkn_bd, 0.0)
        nc.sync.dma_start(out=nkn_bd[0:DH, 0:HEADS:2], in_=nknT[:, 0:HEADS:2])
        nc.sync.dma_start(out=nkn_bd[DH:128, 1:HEADS:2], in_=nknT[:, 1:HEADS:2])

        # null-v: nv_bd2 [2, HEADS, DH+1]; row parity selects head parity
        nv_t = singles.tile([HEADS, DH], f32)
        nc.sync.dma_start(out=nv_t, in_=nv[:, :])
        nvb = singles.tile([HEADS, DH + 1], bf16)
        nc.vector.tensor_copy(out=nvb[:, 0:DH], in_=nv_t)
        nc.vector.memset(nvb[:, DH : DH + 1], 1.0)
        nv_bd2 = singles.tile([2, HEADS, DH + 1], bf16)
        nc.vector.memset(nv_bd2, 0.0)
        nc.sync.dma_start(out=nvb_d[:, :], in_=nvb)
        nc.sync.dma_start(
            out=nv_bd2[0:1, 0:HEADS:2, :],
            in_=nvb_d.ap()[0:HEADS:2, :].partition_broadcast(1),
        )
        nc.sync.dma_start(
            out=nv_bd2[1:2, 1:HEADS:2, :],
            in_=nvb_d.ap()[1:HEADS:2, :].partition_broadcast(1),
        )

        ones_f = singles.tile([1, DH], f32)
        nc.vector.memset(ones_f, 1.0)

        # ---------------- phase 1: LayerNorm ----------------
        with (
            tc.tile_pool(name="px", bufs=3) as px,
            tc.tile_pool(name="pst", bufs=4) as pst,
            tc.tile_pool(name="pxc", bufs=2) as pxc,
            tc.tile_pool(name="pxn", bufs=2) as pxn,
        ):
            for tt in range(NT):
                r0 = tt * 128
                xt = px.tile([128, DIM], f32)
                nc.sync.dma_start(out=xt, in_=x[r0 : r0 + 128, :])
                stats = pst.tile([128, 2, 6], f32, tag="stats")
                nc.vector.bn_stats(out=stats[:, 0, :], in_=xt[:, 0:512])
                nc.vector.bn_stats(out=stats[:, 1, :], in_=xt[:, 512:1024])
                mv = pst.tile([128, 2], f32, tag="mv")
                nc.vector.bn_aggr(out=mv, in_=stats)
                rr = pst.tile([128, 1], f32, tag="rr")
                nc.scalar.activation(out=rr, in_=mv[:, 1:2], func=AF.Ln, bias=eps_t)
                nc.scalar.activation(out=rr, in_=rr, func=AF.Exp, scale=-0.5)
                xc = pxc.tile([128, DIM], f32)
                nc.vector.tensor_scalar(
                    out=xc, in0=xt, scalar1=mv[:, 0:1], scalar2=rr,
                    op0=OP.subtract, op1=OP.mult,
                )
                nc.vector.tensor_tensor(out=xc, in0=xc, in1=gamma_b, op=OP.mult)
                xnt = pxn.tile([128, DIM], bf16)
                nc.vector.tensor_tensor(out=xnt, in0=xc, in1=beta_b, op=OP.add)
                nc.sync.dma_start(out=xn_d[r0 : r0 + 128, :], in_=xnt)

        # ---------------- phase 2: xn^T via DRAM transpose ----------------
        for c in range(NCD):
            nc.sync.dma_start(
                out=xnT[:, c, :], in_=xn_d[:, c * 128 : (c + 1) * 128], transpose=True
            )

        # ---------------- phase 3: projections ----------------
        def load_w_half(pw, pwst, W, half):
            wt = pw.tile([128, NCD, 512], bf16, tag="W")
            for c in range(NCD):
                ws = pwst.tile([128, 512], f32, tag="wstage")
                nc.sync.dma_start(
                    out=ws, in_=W[c * 128 : (c + 1) * 128, half * 512 : (half + 1) * 512]
                )
                nc.vector.tensor_copy(out=wt[:, c, :], in_=ws)
            return wt

        with (
            tc.tile_pool(name="pw", bufs=2) as pw,
            tc.tile_pool(name="pwst", bufs=3) as pwst,
            tc.tile_pool(name="ppj", bufs=3, space="PSUM") as ppj,
            tc.tile_pool(name="pnrm", bufs=3) as pnrm,
            tc.tile_pool(name="pout", bufs=3) as pout,
        ):
            # ---- k projection (all 2048 tokens), l2norm along d, no scale
            for half in range(2):
                wk = load_w_half(pw, pwst, Wk, half)
                for tt in range(NT):
                    r0 = tt * 128
                    kp = ppj.tile([128, 512], f32, tag="pj")
                    for c in range(NCD):
                        nc.tensor.matmul(
                            kp, lhsT=xnT[:, c, r0 : r0 + 128], rhs=wk[:, c, :],
                            start=(c == 0), stop=(c == NCD - 1),
                        )
                    sq = pnrm.tile([128, 512], f32, tag="sq")
                    nc.scalar.activation(out=sq, in_=kp, func=AF.Square)
                    s8 = pnrm.tile([128, 8], f32, tag="s8")
                    nc.vector.tensor_reduce(
                        out=s8, in_=sq.rearrange("p (g d) -> p g d", g=8),
                        axis=AX.X, op=OP.add,
                    )
                    nc.scalar.activation(out=s8, in_=s8, func=AF.Ln, bias=eps30)
                    nc.scalar.activation(out=s8, in_=s8, func=AF.Exp, scale=-0.5)
                    nc.vector.tensor_scalar_min(out=s8, in0=s8, scalar1=1e12)
                    rex = pnrm.tile([128, 8, DH], f32, tag="rex")
                    nc.vector.tensor_copy(out=rex, in_=s8.broadcast_to([128, 8, DH]))
                    knf = pout.tile([128, 512], bf16, tag="knf")
                    nc.vector.tensor_tensor(out=knf, in0=kp, in1=rex, op=OP.mult)
                    nc.sync.dma_start(
                        out=kn_d[r0 : r0 + 128, half * 512 : (half + 1) * 512], in_=knf
                    )

            # ---- q projection (first 1024 tokens), l2norm, * (qs*ks)
            for half in range(2):
                wq = load_w_half(pw, pwst, Wq, half)
                for tt in range(NTQ):
                    r0 = tt * 128
                    qp = ppj.tile([128, 512], f32, tag="pj")
                    for c in range(NCD):
                        nc.tensor.matmul(
                            qp, lhsT=xnT[:, c, r0 : r0 + 128], rhs=wq[:, c, :],
                            start=(c == 0), stop=(c == NCD - 1),
                        )
                    sq = pnrm.tile([128, 512], f32, tag="sq")
                    nc.scalar.activation(out=sq, in_=qp, func=AF.Square)
                    s8 = pnrm.tile([128, 8], f32, tag="s8")
                    nc.vector.tensor_reduce(
                        out=s8, in_=sq.rearrange("p (g d) -> p g d", g=8),
                        axis=AX.X, op=OP.add,
                    )
                    nc.scalar.activation(out=s8, in_=s8, func=AF.Ln, bias=eps30)
                    nc.scalar.activation(out=s8, in_=s8, func=AF.Exp, scale=-0.5)
                    nc.vector.tensor_scalar_min(out=s8, in0=s8, scalar1=1e12)
                    rex = pnrm.tile([128, 8, DH], f32, tag="rex")
                    nc.vector.tensor_copy(out=rex, in_=s8.broadcast_to([128, 8, DH]))
                    qn1 = pnrm.tile([128, 512], f32, tag="qn1")
                    nc.vector.tensor_tensor(out=qn1, in0=qp, in1=rex, op=OP.mult)
                    qnf = pout.tile([128, 512], bf16, tag="knf")
                    nc.vector.tensor_tensor(
                        out=qnf, in0=qn1,
                        in1=c8.rearrange("p g d -> p (g d)"), op=OP.mult,
                    )
                    nc.sync.dma_start(
                        out=qn_d[r0 : r0 + 128, half * 512 : (half + 1) * 512], in_=qnf
                    )

            # ---- k^T / q^T via DRAM transpose
            for p in range(HP):
                nc.sync.dma_start(
                    out=kT[:, p, :], in_=kn_d[:, p * 128 : (p + 1) * 128], transpose=True
                )
                nc.sync.dma_start(
                    out=qT[:, p, :], in_=qn_d[:, p * 128 : (p + 1) * 128], transpose=True
                )

            # ---- v projection (all tokens) -> V' natural layout
            for half in range(2):
                wv = load_w_half(pw, pwst, Wv, half)
                for tt in range(NT):
                    r0 = tt * 128
                    vp = ppj.tile([128, 512], f32, tag="pj")
                    for c in range(NCD):
                        nc.tensor.matmul(
                            vp, lhsT=xnT[:, c, r0 : r0 + 128], rhs=wv[:, c, :],
                            start=(c == 0), stop=(c == NCD - 1),
                        )
                    nc.vector.tensor_copy(
                        out=vsb[:, tt, half * 8 : (half + 1) * 8, 0:DH],
                        in_=vp.rearrange("p (g d) -> p g d", g=8),
                    )

        # ---------------- phase 5: attention ----------------
        QB = NQ // 512  # 2 query blocks of 512
        with (
            tc.tile_pool(name="pstt", bufs=1, space="PSUM") as pstt,
            tc.tile_pool(name="pot", bufs=3, space="PSUM") as pot,
            tc.tile_pool(name="paux", bufs=1, space="PSUM") as paux,
            tc.tile_pool(name="ppt", bufs=2) as ppt,
            tc.tile_pool(name="pptn", bufs=2) as pptn,
            tc.tile_pool(name="prec", bufs=2) as prec,
            tc.tile_pool(name="pbsc", bufs=2) as pbsc,
        ):
            for hp in range(HP):
                hA, hB = 2 * hp, 2 * hp + 1
                for qb in range(QB):
                    q0 = qb * 512
                    # null scores for both heads: [2, 512]
                    null_ps = paux.tile([2, 512], f32, tag="aux")
                    nc.tensor.matmul(
                        null_ps, lhsT=nkn_bd[:, hA : hA + 2],
                        rhs=qT[:, hp, q0 : q0 + 512], start=True, stop=True,
                    )
                    pTn = pptn.tile([2, 512], bf16)
                    nc.scalar.activation(out=pTn, in_=null_ps, func=AF.Exp, scale=SCALE)

                    otA = pot.tile([DH + 1, 512], f32, tag="ot")
                    otB = pot.tile([DH + 1, 512], f32, tag="ot")

                    for g in range(8):
                        c0, c1 = 2 * g, 2 * g + 1
                        st = pstt.tile([128, 4, 512], f32, tag="st")
                        for si, (h, c, rh) in enumerate(
                            ((hA, c0, 0), (hB, c0, 1), (hA, c1, 0), (hB, c1, 1))
                        ):
                            nc.tensor.matmul(
                                st[:, si, :],
                                lhsT=kT[rh * DH : (rh + 1) * DH, hp, c * 128 : (c + 1) * 128],
                                rhs=qT[rh * DH : (rh + 1) * DH, hp, q0 : q0 + 512],
                                start=True, stop=True,
                                tile_position=(rh * DH, 0),
                            )
                        pt = ppt.tile([128, 4, 512], bf16)
                        nc.scalar.activation(out=pt, in_=st, func=AF.Exp, scale=SCALE)
                        for si, (ot, h, c) in enumerate(
                            ((otA, hA, c0), (otB, hB, c0), (otA, hA, c1), (otB, hB, c1))
                        ):
                            nc.tensor.matmul(
                                ot, lhsT=vsb[:, c, h, :], rhs=pt[:, si, :],
                                start=(c == 0), stop=False,
                            )
                    # null PV (finishes accumulation)
                    nc.tensor.matmul(
                        otA, lhsT=nv_bd2[:, hA, :], rhs=pTn, start=False, stop=True
                    )
                    nc.tensor.matmul(
                        otB, lhsT=nv_bd2[:, hB, :], rhs=pTn, start=False, stop=True
                    )
                    # divide by denominator (row DH of ot) and write A^T
                    for h, ot in ((hA, otA), (hB, otB)):
                        rec = prec.tile([1, 512], f32)
                        nc.vector.reciprocal(rec, ot[DH : DH + 1, :])
                        bc = pstt.tile([DH, 512], f32, tag="st")
                        nc.tensor.matmul(bc, lhsT=ones_f, rhs=rec, start=True, stop=True)
                        bcs = pbsc.tile([DH, 512], f32, tag="bcs")
                        nc.vector.tensor_copy(out=bcs, in_=bc)
                        if h % 2 == 0:
                            nc.vector.tensor_tensor(
                                out=AT[0:DH, h // 2, q0 : q0 + 512],
                                in0=ot[0:DH, :], in1=bcs, op=OP.mult,
                            )
                        else:
                            bs = pbsc.tile([DH, 512], bf16)
                            nc.vector.tensor_tensor(
                                out=bs, in0=ot[0:DH, :], in1=bcs, op=OP.mult
                            )
                            nc.sync.dma_start(
                                out=AT[DH:128, h // 2, q0 : q0 + 512], in_=bs
                            )

        # ---------------- phase 6: output projection ----------------
        with (
            tc.tile_pool(name="pw2", bufs=2) as pw2,
            tc.tile_pool(name="pwst2", bufs=3) as pwst2,
            tc.tile_pool(name="ppj2", bufs=3, space="PSUM") as ppj2,
            tc.tile_pool(name="pob", bufs=3) as pob,
        ):
            for half in range(2):
                wo = load_w_half(pw2, pwst2, Wo, half)
                for tt in range(NTQ):
                    r0 = tt * 128
                    op_ = ppj2.tile([128, 512], f32)
                    for c in range(NCD):
                        nc.tensor.matmul(
                            op_, lhsT=AT[:, c, r0 : r0 + 128], rhs=wo[:, c, :],
                            start=(c == 0), stop=(c == NCD - 1),
                        )
                    ob = pob.tile([128, 512], f32)
                    nc.vector.tensor_copy(out=ob, in_=op_)
                    nc.sync.dma_start(
                        out=out[r0 : r0 + 128, half * 512 : (half + 1) * 512], in_=ob
                    )

    nc.compile()
    return nc


def _get_program():
    if "nc" not in _CACHE:
        _CACHE["nc"] = _build_program()
    return _CACHE["nc"]


def kernel(**inputs) -> np.ndarray:
    from concourse.bass_utils import run_bass_kernel_spmd

    nc = _get_program()

    x = np.asarray(inputs["x"], dtype=np.float32)
    gamma = np.asarray(inputs["gamma"], dtype=np.float32)
    beta = np.asarray(inputs["beta"], dtype=np.float32)
    null_kv = np.asarray(inputs["null_kv"], dtype=np.float32)
    Wq = np.ascontiguousarray(np.asarray(inputs["Wq"], dtype=np.float32))
    Wkv = np.asarray(inputs["Wkv"], dtype=np.float32)
    q_scale = np.asarray(inputs["q_scale"], dtype=np.float32)
    k_scale = np.asarray(inputs["k_scale"], dtype=np.float32)
    Wo = np.ascontiguousarray(np.asarray(inputs["Wo"], dtype=np.float32))

    Wk = np.ascontiguousarray(Wkv[:, :INNER])
    Wv = np.ascontiguousarray(Wkv[:, INNER:])
    nk = np.ascontiguousarray(null_kv[0, :, 0, :])
    nv = np.ascontiguousarray(null_kv[1, :, 0, :])

    in_maps = []
    for b in range(B):
        for hi in range(2):
            xb = x[b]
            if hi == 1:
                xb = np.concatenate([xb[NQ:], xb[:NQ]], axis=0)
            in_maps.append(
                {
                    "x": np.ascontiguousarray(xb),
                    "gamma": gamma,
                    "beta": beta,
                    "Wq": Wq,
                    "Wk": Wk,
                    "Wv": Wv,
                    "Wo": Wo,
                    "nk": nk,
                    "nv": nv,
                    "qs": q_scale,
                    "ks": k_scale,
                }
            )

    res = run_bass_kernel_spmd(nc, in_maps, list(range(8)))

    full = np.empty((B, N, DIM), dtype=np.float32)
    for c in range(8):
        b, hi = divmod(c, 2)
        full[b, hi * NQ : (hi + 1) * NQ] = res.results[c]["out"]
    return full


# revision 10
# speedup vs baseline: 1.0146x; 1.0146x over previous
"""Trainium2 Bass kernel for nn_Attention (LayerNorm + L2-normalized-QK attention
with null-kv slot + output projection), SPMD across 8 NeuronCores.

Sharding: core c = (batch b = c//2, query-half hi = c%2). Each core computes the
full kv (2048 tokens) of its batch and attention outputs for its 1024-query
half. Softmax over kv is permutation invariant, so for hi=1 we feed x with the
two sequence halves swapped — every core then runs the identical SPMD program
with its queries in rows 0:1024. The final output is a pure concatenation of
the per-core results (no collectives, no host arithmetic).

Device-side layout choices:
  - S is computed transposed (S^T [kv, q]) so no softmax row-max pass is
    needed: q,k are L2-normalized so |8*q.k| <= 8 and exp() cannot overflow.
  - PV uses V' = [V | 1] (M=65) so the softmax denominator falls out of the
    same matmul chain, and the output lands directly in A^T layout for the
    output projection.
  - rsqrt is computed as exp(-0.5*ln(x)) so the only ACT table set used is
    natural_log_exp_and_others (no table thrashing with the softmax exp).
  - all matmuls in bf16 with fp32 PSUM accumulation.
"""

import numpy as np

B = 4
N = 2048
DIM = 1024
HEADS = 16
DH = 64
INNER = HEADS * DH
NQ = 1024  # queries per core
SCALE = 8.0
LN_EPS = 1e-5

_CACHE = {}


def _build_program():
    from contextlib import ExitStack

    import concourse.bacc as bacc
    import concourse.bass as bass
    import concourse.tile as tile
    from concourse import mybir

    f32 = mybir.dt.float32
    bf16 = mybir.dt.bfloat16
    AF = mybir.ActivationFunctionType
    OP = mybir.AluOpType
    AX = mybir.AxisListType

    NT = N // 128          # 16 token tiles
    NTQ = NQ // 128        # 8 query token tiles
    NCD = DIM // 128       # 8 dim chunks
    HP = HEADS // 2        # 8 head pairs

    nc = bacc.Bacc("TRN2", target_bir_lowering=False, debug=False)

    x = nc.declare_dram_parameter("x", [N, DIM], f32, isOutput=False)
    gamma = nc.declare_dram_parameter("gamma", [DIM], f32, isOutput=False)
    beta = nc.declare_dram_parameter("beta", [DIM], f32, isOutput=False)
    Wq = nc.declare_dram_parameter("Wq", [DIM, INNER], f32, isOutput=False)
    Wk = nc.declare_dram_parameter("Wk", [DIM, INNER], f32, isOutput=False)
    Wv = nc.declare_dram_parameter("Wv", [DIM, INNER], f32, isOutput=False)
    Wo = nc.declare_dram_parameter("Wo", [INNER, DIM], f32, isOutput=False)
    nk = nc.declare_dram_parameter("nk", [HEADS, DH], f32, isOutput=False)
    nv = nc.declare_dram_parameter("nv", [HEADS, DH], f32, isOutput=False)
    qs = nc.declare_dram_parameter("qs", [DH], f32, isOutput=False)
    ks = nc.declare_dram_parameter("ks", [DH], f32, isOutput=False)
    out = nc.declare_dram_parameter("out", [NQ, DIM], f32, isOutput=True)

    # internal DRAM for transpose round-trips
    xn_d = nc.dram_tensor("xn_d", [N, DIM], bf16)
    kn_d = nc.dram_tensor("kn_d", [N, INNER], bf16)
    qn_d = nc.dram_tensor("qn_d", [NQ, INNER], bf16)
    nkn_d = nc.dram_tensor("nkn_d", [HEADS, DH], bf16)
    nvb_d = nc.dram_tensor("nvb_d", [HEADS, DH + 1], bf16)

    with tile.TileContext(nc) as tc, ExitStack() as ctx:
        singles = ctx.enter_context(tc.tile_pool(name="singles", bufs=1))
        big = ctx.enter_context(tc.tile_pool(name="big", bufs=1))

        # ---------------- persistent SBUF tensors ----------------
        xnT = big.tile([128, NCD, N], bf16, tag="xnT")       # xn^T  [dim, tok]
        kT = big.tile([128, HP, N], bf16, tag="kT")          # k^T   [2*64, kv] per pair
        qT = big.tile([128, HP, NQ], bf16, tag="qT")         # q^T
        vsb = big.tile([128, NT, HEADS, DH + 1], bf16, tag="v")   # V'=[V|1]
        AT = big.tile([128, NCD, NQ], bf16, tag="AT")        # A^T (attn out)

        # ---------------- constants ----------------
        gamma_b = singles.tile([128, DIM], f32)
        nc.gpsimd.dma_start(out=gamma_b, in_=gamma.ap().partition_broadcast(128))
        beta_b = singles.tile([128, DIM], f32)
        nc.gpsimd.dma_start(out=beta_b, in_=beta.ap().partition_broadcast(128))
        eps_t = singles.tile([128, 1], f32)
        nc.vector.memset(eps_t, LN_EPS)
        eps30 = singles.tile([128, 1], f32)
        nc.vector.memset(eps30, 1e-30)

        qs_b = singles.tile([128, DH], f32)
        nc.gpsimd.dma_start(out=qs_b, in_=qs.ap().partition_broadcast(128))
        ks_b = singles.tile([128, DH], f32)
        nc.gpsimd.dma_start(out=ks_b, in_=ks.ap().partition_broadcast(128))
        c64 = singles.tile([128, DH], f32)
        nc.vector.tensor_tensor(out=c64, in0=qs_b, in1=ks_b, op=OP.mult)
        c8 = singles.tile([128, 8, DH], f32)   # qs*ks tiled for 8 heads (one col half)
        for g in range(8):
            nc.vector.tensor_copy(out=c8[:, g, :], in_=c64)

        nc.vector.memset(vsb[:, :, :, DH : DH + 1], 1.0)  # ones column of V'

        # null-kv prep: nkn = l2norm(nk)*qs*ks (bf16), then transpose via DRAM
        nk_t = singles.tile([HEADS, DH], f32)
        nc.sync.dma_start(out=nk_t, in_=nk[:, :])
        nksq = singles.tile([HEADS, DH], f32)
        nc.vector.tensor_tensor(out=nksq, in0=nk_t, in1=nk_t, op=OP.mult)
        nks = singles.tile([HEADS, 1], f32)
        nc.vector.tensor_reduce(out=nks, in_=nksq, axis=AX.X, op=OP.add)
        nc.scalar.activation(out=nks, in_=nks, func=AF.Ln, bias=eps30[0:HEADS, :])
        nc.scalar.activation(out=nks, in_=nks, func=AF.Exp, scale=-0.5)
        nc.vector.tensor_scalar_min(out=nks, in0=nks, scalar1=1e12)
        nkn = singles.tile([HEADS, DH], f32)
        nc.vector.tensor_scalar_mul(out=nkn, in0=nk_t, scalar1=nks)
        nknb = singles.tile([HEADS, DH], bf16)
        nc.vector.tensor_tensor(out=nknb, in0=nkn, in1=c64[0:HEADS, :], op=OP.mult)
        nc.sync.dma_start(out=nkn_d[:, :], in_=nknb)
        nknT = singles.tile([DH, HEADS], bf16)
        nc.sync.dma_start(out=nknT, in_=nkn_d.ap().rearrange("h d -> d h"))
        # block-diagonal [128, 16]: col 2p rows 0:64 = head 2p, col 2p+1 rows 64:128 = head 2p+1
        nkn_bd = singles.tile([128, HEADS], bf16)
        nc.vector.memset(nkn_bd, 0.0)
        nc.sync.dma_start(out=nkn_bd[0:DH, 0:HEADS:2], in_=nknT[:, 0:HEADS:2])
        nc.sync.dma_start(out=nkn_bd[DH:128, 1:HEADS:2], in_=nknT[:, 1:HEADS:2])

        # null-v: nv_bd2 [2, HEADS, DH+1]; row parity selects head parity
        nv_t = singles.tile([HEADS, DH], f32)
        nc.sync.dma_start(out=nv_t, in_=nv[:, :])
        nvb = singles.tile([HEADS, DH + 1], bf16)
        nc.vector.tensor_copy(out=nvb[:, 0:DH], in_=nv_t)
        nc.vector.memset(nvb[:, DH : DH + 1], 1.0)
        nv_bd2 = singles.tile([2, HEADS, DH + 1], bf16)
        nc.vector.memset(nv_bd2, 0.0)
        nc.sync.dma_start(out=nvb_d[:, :], in_=nvb)
        nc.sync.dma_start(
            out=nv_bd2[0:1, 0:HEADS:2, :],
            in_=nvb_d.ap()[0:HEADS:2, :].partition_broadcast(1),
        )
        nc.sync.dma_start(
            out=nv_bd2[1:2, 1:HEADS:2, :],
            in_=nvb_d.ap()[1:HEADS:2, :].partition_broadcast(1),
        )

        ones_f = singles.tile([1, DH], f32)
        nc.vector.memset(ones_f, 1.0)

        # ---------------- phase 1: LayerNorm (two passes, batched rsqrt) -----
        mv_all = singles.tile([128, NT, 2], f32)
        rst_all = singles.tile([128, NT], f32)
        with (
            tc.tile_pool(name="px", bufs=2) as px,
            tc.tile_pool(name="pst", bufs=4) as pst,
            tc.tile_pool(name="pxc", bufs=2) as pxc,
            tc.tile_pool(name="pxn", bufs=2) as pxn,
        ):
            for tt in range(NT):
                r0 = tt * 128
                xt = px.tile([128, DIM], f32)
                nc.sync.dma_start(out=xt, in_=x[r0 : r0 + 128, :])
                stats = pst.tile([128, 2, 6], f32, tag="stats")
                nc.vector.bn_stats(out=stats[:, 0, :], in_=xt[:, 0:512])
                nc.vector.bn_stats(out=stats[:, 1, :], in_=xt[:, 512:1024])
                nc.vector.bn_aggr(out=mv_all[:, tt, :], in_=stats)
            # batched rstd = exp(-0.5*ln(var+eps)) -- 1 table pair for all tiles
            nc.scalar.activation(
                out=rst_all, in_=mv_all[:, :, 1], func=AF.Ln, bias=eps_t
            )
            nc.scalar.activation(out=rst_all, in_=rst_all, func=AF.Exp, scale=-0.5)
            for tt in range(NT):
                r0 = tt * 128
                xt = px.tile([128, DIM], f32)
                nc.sync.dma_start(out=xt, in_=x[r0 : r0 + 128, :])
                xc = pxc.tile([128, DIM], f32)
                nc.vector.tensor_scalar(
                    out=xc, in0=xt, scalar1=mv_all[:, tt, 0:1],
                    scalar2=rst_all[:, tt : tt + 1],
                    op0=OP.subtract, op1=OP.mult,
                )
                nc.vector.tensor_tensor(out=xc, in0=xc, in1=gamma_b, op=OP.mult)
                xnt = pxn.tile([128, DIM], bf16)
                nc.vector.tensor_tensor(out=xnt, in0=xc, in1=beta_b, op=OP.add)
                nc.sync.dma_start(out=xn_d[r0 : r0 + 128, :], in_=xnt)

        # ---------------- phase 2: xn^T via DRAM transpose ----------------
        for c in range(NCD):
            nc.sync.dma_start(
                out=xnT[:, c, :], in_=xn_d[:, c * 128 : (c + 1) * 128], transpose=True
            )

        # ---------------- phase 3: projections ----------------
        def load_w_half(pw, pwst, W, half):
            wt = pw.tile([128, NCD, 512], bf16, tag="W")
            for c in range(NCD):
                ws = pwst.tile([128, 512], f32, tag="wstage")
                nc.sync.dma_start(
                    out=ws, in_=W[c * 128 : (c + 1) * 128, half * 512 : (half + 1) * 512]
                )
                nc.vector.tensor_copy(out=wt[:, c, :], in_=ws)
            return wt

        with (
            tc.tile_pool(name="pw", bufs=2) as pw,
            tc.tile_pool(name="pwst", bufs=3) as pwst,
            tc.tile_pool(name="ppj", bufs=3, space="PSUM") as ppj,
            tc.tile_pool(name="pnrm", bufs=2) as pnrm,
            tc.tile_pool(name="pout", bufs=1) as pout,
        ):
            # ---- k projection (all 2048 tokens), l2norm along d, no scale
            for half in range(2):
                wk = load_w_half(pw, pwst, Wk, half)
                s_all = pnrm.tile([128, NT, 8], f32, tag="sall")
                ksb = []
                for tt in range(NT):
                    r0 = tt * 128
                    kp = ppj.tile([128, 512], f32, tag="pj")
                    for c in range(NCD):
                        nc.tensor.matmul(
                            kp, lhsT=xnT[:, c, r0 : r0 + 128], rhs=wk[:, c, :],
                            start=(c == 0), stop=(c == NCD - 1),
                        )
                    sq = pnrm.tile([128, 512], f32, tag="sq")
                    nc.scalar.activation(out=sq, in_=kp, func=AF.Square)
                    nc.vector.tensor_reduce(
                        out=s_all[:, tt, :], in_=sq.rearrange("p (g d) -> p g d", g=8),
                        axis=AX.X, op=OP.add,
                    )
                    ks_t = pout.tile([128, 512], bf16, tag=f"ksb{tt}")
                    nc.scalar.copy(out=ks_t, in_=kp)
                    ksb.append(ks_t)
                # batched rsqrt for the whole half (one Ln/Exp table pair)
                nc.scalar.activation(
                    out=s_all.rearrange("p a b -> p (a b)"),
                    in_=s_all.rearrange("p a b -> p (a b)"), func=AF.Ln, bias=eps30,
                )
                nc.scalar.activation(
                    out=s_all.rearrange("p a b -> p (a b)"),
                    in_=s_all.rearrange("p a b -> p (a b)"), func=AF.Exp, scale=-0.5,
                )
                nc.vector.tensor_scalar_min(
                    out=s_all.rearrange("p a b -> p (a b)"),
                    in0=s_all.rearrange("p a b -> p (a b)"), scalar1=1e12,
                )
                for tt in range(NT):
                    r0 = tt * 128
                    rex = pnrm.tile([128, 8, DH], f32, tag="rex")
                    nc.vector.tensor_copy(
                        out=rex, in_=s_all[:, tt, :].broadcast_to([128, 8, DH])
                    )
                    knf = pnrm.tile([128, 512], bf16, tag="knf")
                    nc.vector.tensor_tensor(out=knf, in0=ksb[tt], in1=rex, op=OP.mult)
                    nc.sync.dma_start(
                        out=kn_d[r0 : r0 + 128, half * 512 : (half + 1) * 512], in_=knf
                    )

            # ---- q projection (first 1024 tokens), l2norm, * (qs*ks)
            last_qnorm_exp = None
            for half in range(2):
                wq = load_w_half(pw, pwst, Wq, half)
                s_all = pnrm.tile([128, NTQ, 8], f32, tag="sallq")
                qsb = []
                for tt in range(NTQ):
                    r0 = tt * 128
                    qp = ppj.tile([128, 512], f32, tag="pj")
                    for c in range(NCD):
                        nc.tensor.matmul(
                            qp, lhsT=xnT[:, c, r0 : r0 + 128], rhs=wq[:, c, :],
                            start=(c == 0), stop=(c == NCD - 1),
                        )
                    sq = pnrm.tile([128, 512], f32, tag="sq")
                    nc.scalar.activation(out=sq, in_=qp, func=AF.Square)
                    nc.vector.tensor_reduce(
                        out=s_all[:, tt, :], in_=sq.rearrange("p (g d) -> p g d", g=8),
                        axis=AX.X, op=OP.add,
                    )
                    qs_t = pout.tile([128, 512], bf16, tag=f"qsb{tt}")
                    nc.scalar.copy(out=qs_t, in_=qp)
                    qsb.append(qs_t)
                nc.scalar.activation(
                    out=s_all.rearrange("p a b -> p (a b)"),
                    in_=s_all.rearrange("p a b -> p (a b)"), func=AF.Ln, bias=eps30,
                )
                last_qnorm_exp = nc.scalar.activation(
                    out=s_all.rearrange("p a b -> p (a b)"),
                    in_=s_all.rearrange("p a b -> p (a b)"), func=AF.Exp, scale=-0.5,
                )
                nc.vector.tensor_scalar_min(
                    out=s_all.rearrange("p a b -> p (a b)"),
                    in0=s_all.rearrange("p a b -> p (a b)"), scalar1=1e12,
                )
                for tt in range(NTQ):
                    r0 = tt * 128
                    rex = pnrm.tile([128, 8, DH], f32, tag="rex")
                    nc.vector.tensor_copy(
                        out=rex, in_=s_all[:, tt, :].broadcast_to([128, 8, DH])
                    )
                    rc = pnrm.tile([128, 512], f32, tag="qn1")
                    nc.vector.tensor_tensor(
                        out=rc, in0=rex.rearrange("p g d -> p (g d)"),
                        in1=c8.rearrange("p g d -> p (g d)"), op=OP.mult,
                    )
                    qnf = pnrm.tile([128, 512], bf16, tag="knf")
                    nc.vector.tensor_tensor(out=qnf, in0=qsb[tt], in1=rc, op=OP.mult)
                    nc.sync.dma_start(
                        out=qn_d[r0 : r0 + 128, half * 512 : (half + 1) * 512], in_=qnf
                    )

            # ---- k^T / q^T via DRAM transpose
            for p in range(HP):
                nc.sync.dma_start(
                    out=kT[:, p, :], in_=kn_d[:, p * 128 : (p + 1) * 128], transpose=True
                )
                nc.sync.dma_start(
                    out=qT[:, p, :], in_=qn_d[:, p * 128 : (p + 1) * 128], transpose=True
                )

            # ---- v projection (all tokens) -> V' natural layout
            for half in range(2):
                wv = load_w_half(pw, pwst, Wv, half)
                for tt in range(NT):
                    r0 = tt * 128
                    vp = ppj.tile([128, 512], f32, tag="pj")
                    for c in range(NCD):
                        nc.tensor.matmul(
                            vp, lhsT=xnT[:, c, r0 : r0 + 128], rhs=wv[:, c, :],
                            start=(c == 0), stop=(c == NCD - 1),
                        )
                    nc.vector.tensor_copy(
                        out=vsb[:, tt, half * 8 : (half + 1) * 8, 0:DH],
                        in_=vp.rearrange("p (g d) -> p g d", g=8),
                    )

        # ---------------- phase 5: attention ----------------
        QB = NQ // 512  # 2 query blocks of 512
        with (
            tc.tile_pool(name="pstt", bufs=1, space="PSUM") as pstt,
            tc.tile_pool(name="pot", bufs=3, space="PSUM") as pot,
            tc.tile_pool(name="paux", bufs=1, space="PSUM") as paux,
            tc.tile_pool(name="ppt", bufs=2) as ppt,
            tc.tile_pool(name="pptn", bufs=2) as pptn,
            tc.tile_pool(name="prec", bufs=2) as prec,
            tc.tile_pool(name="pbsc", bufs=2) as pbsc,
        ):
            first_attn_exp = None
            for hp in range(HP):
                hA, hB = 2 * hp, 2 * hp + 1
                for qb in range(QB):
                    q0 = qb * 512
                    # null scores for both heads: [2, 512]
                    null_ps = paux.tile([2, 512], f32, tag="aux")
                    nc.tensor.matmul(
                        null_ps, lhsT=nkn_bd[:, hA : hA + 2],
                        rhs=qT[:, hp, q0 : q0 + 512], start=True, stop=True,
                    )
                    pTn = pptn.tile([2, 512], bf16)
                    e = nc.scalar.activation(
                        out=pTn, in_=null_ps, func=AF.Exp, scale=SCALE
                    )
                    if first_attn_exp is None:
                        first_attn_exp = e
                        if last_qnorm_exp is not None:
                            tile.add_dep_helper(
                                first_attn_exp.ins, last_qnorm_exp.ins,
                                reason="keep ACT exp phase after all Ln/Exp batches",
                            )

                    otA = pot.tile([DH + 1, 512], f32, tag="ot")
                    otB = pot.tile([DH + 1, 512], f32, tag="ot")

                    for g in range(8):
                        c0, c1 = 2 * g, 2 * g + 1
                        st = pstt.tile([128, 4, 512], f32, tag="st")
                        for si, (h, c, rh) in enumerate(
                            ((hA, c0, 0), (hB, c0, 1), (hA, c1, 0), (hB, c1, 1))
                        ):
                            nc.tensor.matmul(
                                st[:, si, :],
                                lhsT=kT[rh * DH : (rh + 1) * DH, hp, c * 128 : (c + 1) * 128],
                                rhs=qT[rh * DH : (rh + 1) * DH, hp, q0 : q0 + 512],
                                start=True, stop=True,
                                tile_position=(rh * DH, 0),
                            )
                        pt = ppt.tile([128, 4, 512], bf16)
                        nc.scalar.activation(out=pt, in_=st, func=AF.Exp, scale=SCALE)
                        for si, (ot, h, c) in enumerate(
                            ((otA, hA, c0), (otB, hB, c0), (otA, hA, c1), (otB, hB, c1))
                        ):
                            nc.tensor.matmul(
                                ot, lhsT=vsb[:, c, h, :], rhs=pt[:, si, :],
                                start=(c == 0), stop=False,
                            )
                    # null PV (finishes accumulation)
                    nc.tensor.matmul(
                        otA, lhsT=nv_bd2[:, hA, :], rhs=pTn, start=False, stop=True
                    )
                    nc.tensor.matmul(
                        otB, lhsT=nv_bd2[:, hB, :], rhs=pTn, start=False, stop=True
                    )
                    # divide by denominator (row DH of ot) and write A^T
                    for h, ot in ((hA, otA), (hB, otB)):
                        den_s = prec.tile([1, 512], f32)
                        nc.scalar.copy(out=den_s, in_=ot[DH : DH + 1, :])
                        bc = pstt.tile([DH, 512], f32, tag="st")
                        nc.tensor.matmul(
                            bc, lhsT=ones_f, rhs=den_s, start=True, stop=True
                        )
                        rcs = pbsc.tile([DH, 512], f32, tag="bcs")
                        nc.vector.reciprocal(rcs, bc)
                        if h % 2 == 0:
                            nc.vector.tensor_tensor(
                                out=AT[0:DH, h // 2, q0 : q0 + 512],
                                in0=ot[0:DH, :], in1=rcs, op=OP.mult,
                            )
                        else:
                            bs = pbsc.tile([DH, 512], bf16)
                            nc.vector.tensor_tensor(
                                out=bs, in0=ot[0:DH, :], in1=rcs, op=OP.mult
                            )
                            nc.sync.dma_start(
                                out=AT[DH:128, h // 2, q0 : q0 + 512], in_=bs
                            )

        # ---------------- phase 6: output projection ----------------
        with (
            tc.tile_pool(name="pw2", bufs=2) as pw2,
            tc.tile_pool(name="pwst2", bufs=3) as pwst2,
            tc.tile_pool(name="ppj2", bufs=3, space="PSUM") as ppj2,
            tc.tile_pool(name="pob", bufs=3) as pob,
        ):
            for half in range(2):
                wo = load_w_half(pw2, pwst2, Wo, half)
                for tt in range(NTQ):
                    r0 = tt * 128
                    op_ = ppj2.tile([128, 512], f32)
                    for c in range(NCD):
                        nc.tensor.matmul(
                            op_, lhsT=AT[:, c, r0 : r0 + 128], rhs=wo[:, c, :],
                            start=(c == 0), stop=(c == NCD - 1),
                        )
                    ob = pob.tile([128, 512], f32)
                    nc.vector.tensor_copy(out=ob, in_=op_)
                    nc.sync.dma_start(
                        out=out[r0 : r0 + 128, half * 512 : (half + 1) * 512], in_=ob
                    )

    nc.compile()
    return nc


def _get_program():
    if "nc" not in _CACHE:
        _CACHE["nc"] = _build_program()
    return _CACHE["nc"]


def kernel(**inputs) -> np.ndarray:
    from concourse.bass_utils import run_bass_kernel_spmd

    nc = _get_program()

    x = np.asarray(inputs["x"], dtype=np.float32)
    gamma = np.asarray(inputs["gamma"], dtype=np.float32)
    beta = np.asarray(inputs["beta"], dtype=np.float32)
    null_kv = np.asarray(inputs["null_kv"], dtype=np.float32)
    Wq = np.ascontiguousarray(np.asarray(inputs["Wq"], dtype=np.float32))
    Wkv = np.asarray(inputs["Wkv"], dtype=np.float32)
    q_scale = np.asarray(inputs["q_scale"], dtype=np.float32)
    k_scale = np.asarray(inputs["k_scale"], dtype=np.float32)
    Wo = np.ascontiguousarray(np.asarray(inputs["Wo"], dtype=np.float32))

    Wk = np.ascontiguousarray(Wkv[:, :INNER])
    Wv = np.ascontiguousarray(Wkv[:, INNER:])
    nk = np.ascontiguousarray(null_kv[0, :, 0, :])
    nv = np.ascontiguousarray(null_kv[1, :, 0, :])

    in_maps = []
    for b in range(B):
        for hi in range(2):
            xb = x[b]
            if hi == 1:
                xb = np.concatenate([xb[NQ:], xb[:NQ]], axis=0)
            in_maps.append(
                {
                    "x": np.ascontiguousarray(xb),
                    "gamma": gamma,
                    "beta": beta,
                    "Wq": Wq,
                    "Wk": Wk,
                    "Wv": Wv,
                    "Wo": Wo,
                    "nk": nk,
                    "nv": nv,
                    "qs": q_scale,
                    "ks": k_scale,
                }
            )

    res = run_bass_kernel_spmd(nc, in_maps, list(range(8)))

    full = np.empty((B, N, DIM), dtype=np.float32)
    for c in range(8):
        b, hi = divmod(c, 2)
        full[b, hi * NQ : (hi + 1) * NQ] = res.results[c]["out"]
    return full


# revision 16
# speedup vs baseline: 1.3001x; 1.2814x over previous
"""Trainium2 Bass kernel for nn_Attention (LayerNorm + L2-normalized-QK attention
with null-kv slot + output projection), SPMD across 8 NeuronCores.

Sharding: core c = (batch b = c//2, query-half hi = c%2). Each core computes the
full kv (2048 tokens) of its batch and attention outputs for its 1024-query
half. Softmax over kv is permutation invariant, so for hi=1 we feed x with the
two sequence halves swapped — every core then runs the identical SPMD program
with its queries in rows 0:1024. The final output is a pure concatenation of
the per-core results (no collectives, no host arithmetic).

Device-side layout choices:
  - S is computed transposed (S^T [kv, q]) so no softmax row-max pass is
    needed: q,k are L2-normalized so |8*q.k| <= 8 and exp() cannot overflow.
  - PV uses V' = [V | 1] (M=65) so the softmax denominator falls out of the
    same matmul chain, and the output lands directly in A^T layout for the
    output projection.
  - rsqrt is computed as exp(-0.5*ln(x)) so the only ACT table set used is
    natural_log_exp_and_others (no table thrashing with the softmax exp).
  - all matmuls in bf16 with fp32 PSUM accumulation.
"""

import numpy as np

B = 4
N = 2048
DIM = 1024
HEADS = 16
DH = 64
INNER = HEADS * DH
NQ = 1024  # queries per core
SCALE = 8.0
LN_EPS = 1e-5

_CACHE = {}


def _build_program():
    from contextlib import ExitStack

    import concourse.bacc as bacc
    import concourse.bass as bass
    import concourse.tile as tile
    from concourse import mybir

    f32 = mybir.dt.float32
    bf16 = mybir.dt.bfloat16
    AF = mybir.ActivationFunctionType
    OP = mybir.AluOpType
    AX = mybir.AxisListType

    NT = N // 128          # 16 token tiles
    NTQ = NQ // 128        # 8 query token tiles
    NCD = DIM // 128       # 8 dim chunks
    HP = HEADS // 2        # 8 head pairs

    nc = bacc.Bacc("TRN2", target_bir_lowering=False, debug=False)

    x = nc.declare_dram_parameter("x", [N, DIM], f32, isOutput=False)
    gamma = nc.declare_dram_parameter("gamma", [DIM], f32, isOutput=False)
    beta = nc.declare_dram_parameter("beta", [DIM], f32, isOutput=False)
    Wq = nc.declare_dram_parameter("Wq", [DIM, INNER], f32, isOutput=False)
    Wk = nc.declare_dram_parameter("Wk", [DIM, INNER], f32, isOutput=False)
    Wv = nc.declare_dram_parameter("Wv", [DIM, INNER], f32, isOutput=False)
    Wo = nc.declare_dram_parameter("Wo", [INNER, DIM], f32, isOutput=False)
    nk = nc.declare_dram_parameter("nk", [HEADS, DH], f32, isOutput=False)
    nv = nc.declare_dram_parameter("nv", [HEADS, DH], f32, isOutput=False)
    qs = nc.declare_dram_parameter("qs", [DH], f32, isOutput=False)
    ks = nc.declare_dram_parameter("ks", [DH], f32, isOutput=False)
    out = nc.declare_dram_parameter("out", [NQ, DIM], f32, isOutput=True)

    # internal DRAM for transpose round-trips
    xn_d = nc.dram_tensor("xn_d", [N, DIM], bf16)
    kn_d = nc.dram_tensor("kn_d", [N, INNER], bf16)
    qn_d = nc.dram_tensor("qn_d", [NQ, INNER], bf16)
    nkn_d = nc.dram_tensor("nkn_d", [HEADS, DH], bf16)
    nvb_d = nc.dram_tensor("nvb_d", [HEADS, DH + 1], bf16)
    rcp_d = nc.dram_tensor("rcp_d", [HEADS, 2, DH, 8], f32)
    den_d = nc.dram_tensor("den_d", [HEADS, 2, DH, 8], f32)

    with tile.TileContext(nc) as tc, ExitStack() as ctx:
        singles = ctx.enter_context(tc.tile_pool(name="singles", bufs=1))
        big = ctx.enter_context(tc.tile_pool(name="big", bufs=1))

        # ---------------- persistent SBUF tensors ----------------
        xnT = big.tile([128, NCD, N], bf16, tag="xnT")       # xn^T  [dim, tok]
        kT = big.tile([128, HP, N], bf16, tag="kT")          # k^T   [2*64, kv] per pair
        qT = big.tile([128, HP, NQ], bf16, tag="qT")         # q^T
        vsb = big.tile([128, NT, HEADS, DH + 1], bf16, tag="v")   # V'=[V|1]
        AT = big.tile([128, NCD, NQ], bf16, tag="AT")        # A^T (attn out)

        # ---------------- constants ----------------
        gamma_b = singles.tile([128, DIM], f32)
        nc.gpsimd.dma_start(out=gamma_b, in_=gamma.ap().partition_broadcast(128))
        beta_b = singles.tile([128, DIM], f32)
        nc.gpsimd.dma_start(out=beta_b, in_=beta.ap().partition_broadcast(128))
        eps_t = singles.tile([128, 1], f32)
        nc.vector.memset(eps_t, LN_EPS)
        eps30 = singles.tile([128, 1], f32)
        nc.vector.memset(eps30, 1e-30)

        qs_b = singles.tile([128, DH], f32)
        nc.gpsimd.dma_start(out=qs_b, in_=qs.ap().partition_broadcast(128))
        ks_b = singles.tile([128, DH], f32)
        nc.gpsimd.dma_start(out=ks_b, in_=ks.ap().partition_broadcast(128))
        c64 = singles.tile([128, DH], f32)
        nc.vector.tensor_tensor(out=c64, in0=qs_b, in1=ks_b, op=OP.mult)
        c8 = singles.tile([128, 8, DH], f32)   # qs*ks tiled for 8 heads (one col half)
        for g in range(8):
            nc.vector.tensor_copy(out=c8[:, g, :], in_=c64)

        nc.vector.memset(vsb[:, :, :, DH : DH + 1], 1.0)  # ones column of V'

        # null-kv prep: nkn = l2norm(nk)*qs*ks (bf16), then transpose via DRAM
        nk_t = singles.tile([HEADS, DH], f32)
        nc.sync.dma_start(out=nk_t, in_=nk[:, :])
        nksq = singles.tile([HEADS, DH], f32)
        nc.vector.tensor_tensor(out=nksq, in0=nk_t, in1=nk_t, op=OP.mult)
        nks = singles.tile([HEADS, 1], f32)
        nc.vector.tensor_reduce(out=nks, in_=nksq, axis=AX.X, op=OP.add)
        nc.scalar.activation(out=nks, in_=nks, func=AF.Ln, bias=eps30[0:HEADS, :])
        nc.scalar.activation(out=nks, in_=nks, func=AF.Exp, scale=-0.5)
        nc.vector.tensor_scalar_min(out=nks, in0=nks, scalar1=1e12)
        nkn = singles.tile([HEADS, DH], f32)
        nc.vector.tensor_scalar_mul(out=nkn, in0=nk_t, scalar1=nks)
        nknb = singles.tile([HEADS, DH], bf16)
        nc.vector.tensor_tensor(out=nknb, in0=nkn, in1=c64[0:HEADS, :], op=OP.mult)
        nc.sync.dma_start(out=nkn_d[:, :], in_=nknb)
        nknT = singles.tile([DH, HEADS], bf16)
        nc.sync.dma_start(out=nknT, in_=nkn_d.ap().rearrange("h d -> d h"))
        # block-diagonal [128, 16]: col 2p rows 0:64 = head 2p, col 2p+1 rows 64:128 = head 2p+1
        nkn_bd = singles.tile([128, HEADS], bf16)
        nc.vector.memset(nkn_bd, 0.0)
        nc.sync.dma_start(out=nkn_bd[0:DH, 0:HEADS:2], in_=nknT[:, 0:HEADS:2])
        nc.sync.dma_start(out=nkn_bd[DH:128, 1:HEADS:2], in_=nknT[:, 1:HEADS:2])

        # null-v: nv_bd2 [2, HEADS, DH+1]; row parity selects head parity
        nv_t = singles.tile([HEADS, DH], f32)
        nc.sync.dma_start(out=nv_t, in_=nv[:, :])
        nvb = singles.tile([HEADS, DH + 1], bf16)
        nc.vector.tensor_copy(out=nvb[:, 0:DH], in_=nv_t)
        nc.vector.memset(nvb[:, DH : DH + 1], 1.0)
        nv_bd2 = singles.tile([2, HEADS, DH + 1], bf16)
        nc.vector.memset(nv_bd2, 0.0)
        nc.sync.dma_start(out=nvb_d[:, :], in_=nvb)
        nc.sync.dma_start(
            out=nv_bd2[0:1, 0:HEADS:2, :],
            in_=nvb_d.ap()[0:HEADS:2, :].partition_broadcast(1),
        )
        nc.sync.dma_start(
            out=nv_bd2[1:2, 1:HEADS:2, :],
            in_=nvb_d.ap()[1:HEADS:2, :].partition_broadcast(1),
        )



        # ---------------- phase 1: LayerNorm (two passes, batched rsqrt) -----
        mv_all = singles.tile([128, NT, 2], f32)
        rst_all = singles.tile([128, NT], f32)
        with (
            tc.tile_pool(name="px", bufs=2) as px,
            tc.tile_pool(name="pst", bufs=4) as pst,
            tc.tile_pool(name="pxc", bufs=2) as pxc,
            tc.tile_pool(name="pxn", bufs=2) as pxn,
        ):
            for tt in range(NT):
                r0 = tt * 128
                xt = px.tile([128, DIM], f32)
                nc.sync.dma_start(out=xt, in_=x[r0 : r0 + 128, :])
                stats = pst.tile([128, 2, 6], f32, tag="stats")
                nc.vector.bn_stats(out=stats[:, 0, :], in_=xt[:, 0:512])
                nc.vector.bn_stats(out=stats[:, 1, :], in_=xt[:, 512:1024])
                nc.vector.bn_aggr(out=mv_all[:, tt, :], in_=stats)
            # batched rstd = exp(-0.5*ln(var+eps)) -- 1 table pair for all tiles
            nc.scalar.activation(
                out=rst_all, in_=mv_all[:, :, 1], func=AF.Ln, bias=eps_t
            )
            nc.scalar.activation(out=rst_all, in_=rst_all, func=AF.Exp, scale=-0.5)
            for tt in range(NT):
                r0 = tt * 128
                xt = px.tile([128, DIM], f32)
                nc.sync.dma_start(out=xt, in_=x[r0 : r0 + 128, :])
                xc = pxc.tile([128, DIM], f32)
                nc.vector.tensor_scalar(
                    out=xc, in0=xt, scalar1=mv_all[:, tt, 0:1],
                    scalar2=rst_all[:, tt : tt + 1],
                    op0=OP.subtract, op1=OP.mult,
                )
                nc.vector.tensor_tensor(out=xc, in0=xc, in1=gamma_b, op=OP.mult)
                xnt = pxn.tile([128, DIM], bf16)
                nc.vector.tensor_tensor(out=xnt, in0=xc, in1=beta_b, op=OP.add)
                nc.sync.dma_start(out=xn_d[r0 : r0 + 128, :], in_=xnt)

        # ---------------- phase 2: xn^T via DRAM transpose ----------------
        for c in range(NCD):
            nc.sync.dma_start(
                out=xnT[:, c, :], in_=xn_d[:, c * 128 : (c + 1) * 128], transpose=True
            )

        # ---------------- phase 3: projections ----------------
        def load_w_half(pw, pwst, W, half):
            wt = pw.tile([128, NCD, 512], bf16, tag="W")
            for c in range(NCD):
                ws = pwst.tile([128, 512], f32, tag="wstage")
                nc.sync.dma_start(
                    out=ws, in_=W[c * 128 : (c + 1) * 128, half * 512 : (half + 1) * 512]
                )
                nc.vector.tensor_copy(out=wt[:, c, :], in_=ws)
            return wt

        with (
            tc.tile_pool(name="pw", bufs=2) as pw,
            tc.tile_pool(name="pwst", bufs=3) as pwst,
            tc.tile_pool(name="ppj", bufs=3, space="PSUM") as ppj,
            tc.tile_pool(name="pnrm", bufs=2) as pnrm,
            tc.tile_pool(name="pout", bufs=1) as pout,
        ):
            # ---- k projection (all 2048 tokens), l2norm along d, no scale
            for half in range(2):
                wk = load_w_half(pw, pwst, Wk, half)
                s_all = pnrm.tile([128, NT, 8], f32, tag="sall")
                ksb = []
                for tt in range(NT):
                    r0 = tt * 128
                    kp = ppj.tile([128, 512], f32, tag="pj")
                    for c in range(NCD):
                        nc.tensor.matmul(
                            kp, lhsT=xnT[:, c, r0 : r0 + 128], rhs=wk[:, c, :],
                            start=(c == 0), stop=(c == NCD - 1),
                        )
                    sq = pnrm.tile([128, 512], f32, tag="sq")
                    nc.scalar.activation(out=sq, in_=kp, func=AF.Square)
                    nc.vector.tensor_reduce(
                        out=s_all[:, tt, :], in_=sq.rearrange("p (g d) -> p g d", g=8),
                        axis=AX.X, op=OP.add,
                    )
                    ks_t = pout.tile([128, 512], bf16, tag=f"ksb{tt}")
                    nc.vector.tensor_copy(out=ks_t, in_=kp)
                    ksb.append(ks_t)
                # batched rsqrt for the whole half (one Ln/Exp table pair)
                nc.scalar.activation(
                    out=s_all.rearrange("p a b -> p (a b)"),
                    in_=s_all.rearrange("p a b -> p (a b)"), func=AF.Ln, bias=eps30,
                )
                nc.scalar.activation(
                    out=s_all.rearrange("p a b -> p (a b)"),
                    in_=s_all.rearrange("p a b -> p (a b)"), func=AF.Exp, scale=-0.5,
                )
                nc.vector.tensor_scalar_min(
                    out=s_all.rearrange("p a b -> p (a b)"),
                    in0=s_all.rearrange("p a b -> p (a b)"), scalar1=1e12,
                )
                for tt in range(NT):
                    r0 = tt * 128
                    rex = pnrm.tile([128, 8, DH], f32, tag="rex")
                    nc.vector.tensor_copy(
                        out=rex, in_=s_all[:, tt, :].broadcast_to([128, 8, DH])
                    )
                    knf = pnrm.tile([128, 512], bf16, tag="knf")
                    nc.vector.tensor_tensor(out=knf, in0=ksb[tt], in1=rex, op=OP.mult)
                    nc.sync.dma_start(
                        out=kn_d[r0 : r0 + 128, half * 512 : (half + 1) * 512], in_=knf
                    )

            # ---- q projection (first 1024 tokens), l2norm, * (qs*ks)
            last_qnorm_exp = None
            for half in range(2):
                wq = load_w_half(pw, pwst, Wq, half)
                s_all = pnrm.tile([128, NTQ, 8], f32, tag="sallq")
                qsb = []
                for tt in range(NTQ):
                    r0 = tt * 128
                    qp = ppj.tile([128, 512], f32, tag="pj")
                    for c in range(NCD):
                        nc.tensor.matmul(
                            qp, lhsT=xnT[:, c, r0 : r0 + 128], rhs=wq[:, c, :],
                            start=(c == 0), stop=(c == NCD - 1),
                        )
                    sq = pnrm.tile([128, 512], f32, tag="sq")
                    nc.scalar.activation(out=sq, in_=qp, func=AF.Square)
                    nc.vector.tensor_reduce(
                        out=s_all[:, tt, :], in_=sq.rearrange("p (g d) -> p g d", g=8),
                        axis=AX.X, op=OP.add,
                    )
                    qs_t = pout.tile([128, 512], bf16, tag=f"qsb{tt}")
                    nc.vector.tensor_copy(out=qs_t, in_=qp)
                    qsb.append(qs_t)
                nc.scalar.activation(
                    out=s_all.rearrange("p a b -> p (a b)"),
                    in_=s_all.rearrange("p a b -> p (a b)"), func=AF.Ln, bias=eps30,
                )
                last_qnorm_exp = nc.scalar.activation(
                    out=s_all.rearrange("p a b -> p (a b)"),
                    in_=s_all.rearrange("p a b -> p (a b)"), func=AF.Exp, scale=-0.5,
                )
                nc.vector.tensor_scalar_min(
                    out=s_all.rearrange("p a b -> p (a b)"),
                    in0=s_all.rearrange("p a b -> p (a b)"), scalar1=1e12,
                )
                for tt in range(NTQ):
                    r0 = tt * 128
                    rex = pnrm.tile([128, 8, DH], f32, tag="rex")
                    nc.vector.tensor_copy(
                        out=rex, in_=s_all[:, tt, :].broadcast_to([128, 8, DH])
                    )
                    rc = pnrm.tile([128, 512], f32, tag="qn1")
                    nc.vector.tensor_tensor(
                        out=rc, in0=rex.rearrange("p g d -> p (g d)"),
                        in1=c8.rearrange("p g d -> p (g d)"), op=OP.mult,
                    )
                    qnf = pnrm.tile([128, 512], bf16, tag="knf")
                    nc.vector.tensor_tensor(out=qnf, in0=qsb[tt], in1=rc, op=OP.mult)
                    nc.sync.dma_start(
                        out=qn_d[r0 : r0 + 128, half * 512 : (half + 1) * 512], in_=qnf
                    )

            # ---- k^T / q^T via DRAM transpose
            for p in range(HP):
                nc.sync.dma_start(
                    out=kT[:, p, :], in_=kn_d[:, p * 128 : (p + 1) * 128], transpose=True
                )
                nc.sync.dma_start(
                    out=qT[:, p, :], in_=qn_d[:, p * 128 : (p + 1) * 128], transpose=True
                )

            # ---- v projection (all tokens) -> V' natural layout
            for half in range(2):
                wv = load_w_half(pw, pwst, Wv, half)
                for tt in range(NT):
                    r0 = tt * 128
                    vp = ppj.tile([128, 512], f32, tag="pj")
                    for c in range(NCD):
                        nc.tensor.matmul(
                            vp, lhsT=xnT[:, c, r0 : r0 + 128], rhs=wv[:, c, :],
                            start=(c == 0), stop=(c == NCD - 1),
                        )
                    nc.vector.tensor_copy(
                        out=vsb[:, tt, half * 8 : (half + 1) * 8, 0:DH],
                        in_=vp.rearrange("p (g d) -> p g d", g=8),
                    )

        # ---------------- phase 5: attention ----------------
        QB = NQ // 512  # 2 query blocks of 512
        with (
            tc.tile_pool(name="pstt", bufs=2, space="PSUM") as pstt,
            tc.tile_pool(name="pot", bufs=3, space="PSUM") as pot,
            tc.tile_pool(name="paux", bufs=1, space="PSUM") as paux,
            tc.tile_pool(name="ppt", bufs=3) as ppt,
            tc.tile_pool(name="pptn", bufs=2) as pptn,
            tc.tile_pool(name="prec", bufs=2) as prec,
            tc.tile_pool(name="pbsc", bufs=2) as pbsc,
        ):
            first_attn_exp = None
            for hp in range(HP):
                hA, hB = 2 * hp, 2 * hp + 1
                for qb in range(QB):
                    q0 = qb * 512
                    # null scores for both heads: [2, 512]
                    null_ps = paux.tile([2, 512], f32, tag="aux")
                    nc.tensor.matmul(
                        null_ps, lhsT=nkn_bd[:, hA : hA + 2],
                        rhs=qT[:, hp, q0 : q0 + 512], start=True, stop=True,
                    )
                    pTn = pptn.tile([2, 512], bf16)
                    e = nc.scalar.activation(
                        out=pTn, in_=null_ps, func=AF.Exp, scale=SCALE
                    )
                    if first_attn_exp is None:
                        first_attn_exp = e
                        if last_qnorm_exp is not None:
                            tile.add_dep_helper(
                                first_attn_exp.ins, last_qnorm_exp.ins,
                                reason="keep ACT exp phase after all Ln/Exp batches",
                            )

                    otA = pot.tile([DH + 1, 512], f32, tag="ot")
                    otB = pot.tile([DH + 1, 512], f32, tag="ot")

                    for c in range(16):
                        st = pstt.tile([128, 2, 512], f32, tag="st")
                        for si, (h, rh) in enumerate(((hA, 0), (hB, 1))):
                            nc.tensor.matmul(
                                st[:, si, :],
                                lhsT=kT[rh * DH : (rh + 1) * DH, hp, c * 128 : (c + 1) * 128],
                                rhs=qT[rh * DH : (rh + 1) * DH, hp, q0 : q0 + 512],
                                start=True, stop=True,
                                tile_position=(rh * DH, 0),
                            )
                        pt = ppt.tile([128, 2, 512], bf16)
                        nc.scalar.activation(out=pt, in_=st, func=AF.Exp, scale=SCALE)
                        for si, (ot, h) in enumerate(((otA, hA), (otB, hB))):
                            nc.tensor.matmul(
                                ot, lhsT=vsb[:, c, h, :], rhs=pt[:, si, :],
                                start=(c == 0), stop=False,
                            )
                    # null PV (finishes accumulation)
                    nc.tensor.matmul(
                        otA, lhsT=nv_bd2[:, hA, :], rhs=pTn, start=False, stop=True
                    )
                    nc.tensor.matmul(
                        otB, lhsT=nv_bd2[:, hB, :], rhs=pTn, start=False, stop=True
                    )
                    # divide by denominator (row DH of ot) and write A^T
                    for h, ot in ((hA, otA), (hB, otB)):
                        den_s = prec.tile([1, 512], f32, tag="dens")
                        nc.vector.tensor_copy(out=den_s, in_=ot[DH : DH + 1, :])
                        nc.sync.dma_start(
                            out=den_d[h, qb].rearrange("a b -> (a b)").partition_broadcast(1),
                            in_=den_s,
                        )
                        dd = prec.tile([DH, 8], f32, tag="dd")
                        nc.sync.dma_start(out=dd, in_=den_d[h, qb])
                        rr = prec.tile([DH, 8], f32, tag="rr")
                        nc.vector.reciprocal(rr, dd)
                        nc.sync.dma_start(out=rcp_d[h, qb], in_=rr)
                        rcs = pbsc.tile([DH, 512], f32, tag="bcs")
                        nc.sync.dma_start(
                            out=rcs,
                            in_=rcp_d[h, qb].rearrange("a b -> (a b)").partition_broadcast(DH),
                        )
                        po = (h % 2) * DH
                        nc.vector.tensor_tensor(
                            out=AT[po : po + DH, h // 2, q0 : q0 + 512],
                            in0=ot[0:DH, :], in1=rcs, op=OP.mult,
                        )

        # ---------------- phase 6: output projection ----------------
        with (
            tc.tile_pool(name="pw2", bufs=2) as pw2,
            tc.tile_pool(name="pwst2", bufs=3) as pwst2,
            tc.tile_pool(name="ppj2", bufs=3, space="PSUM") as ppj2,
            tc.tile_pool(name="pob", bufs=3) as pob,
        ):
            for half in range(2):
                wo = load_w_half(pw2, pwst2, Wo, half)
                for tt in range(NTQ):
                    r0 = tt * 128
                    op_ = ppj2.tile([128, 512], f32)
                    for c in range(NCD):
                        nc.tensor.matmul(
                            op_, lhsT=AT[:, c, r0 : r0 + 128], rhs=wo[:, c, :],
                            start=(c == 0), stop=(c == NCD - 1),
                        )
                    ob = pob.tile([128, 512], f32)
                    nc.vector.tensor_copy(out=ob, in_=op_)
                    nc.sync.dma_start(
                        out=out[r0 : r0 + 128, half * 512 : (half + 1) * 512], in_=ob
                    )

    nc.compile()
    return nc


def _get_program():
    if "nc" not in _CACHE:
        _CACHE["nc"] = _build_program()
    return _CACHE["nc"]


def kernel(**inputs) -> np.ndarray:
    from concourse.bass_utils import run_bass_kernel_spmd

    nc = _get_program()

    x = np.asarray(inputs["x"], dtype=np.float32)
    gamma = np.asarray(inputs["gamma"], dtype=np.float32)
    beta = np.asarray(inputs["beta"], dtype=np.float32)
    null_kv = np.asarray(inputs["null_kv"], dtype=np.float32)
    Wq = np.ascontiguousarray(np.asarray(inputs["Wq"], dtype=np.float32))
    Wkv = np.asarray(inputs["Wkv"], dtype=np.float32)
    q_scale = np.asarray(inputs["q_scale"], dtype=np.float32)
    k_scale = np.asarray(inputs["k_scale"], dtype=np.float32)
    Wo = np.ascontiguousarray(np.asarray(inputs["Wo"], dtype=np.float32))

    Wk = np.ascontiguousarray(Wkv[:, :INNER])
    Wv = np.ascontiguousarray(Wkv[:, INNER:])
    nk = np.ascontiguousarray(null_kv[0, :, 0, :])
    nv = np.ascontiguousarray(null_kv[1, :, 0, :])

    in_maps = []
    for b in range(B):
        for hi in range(2):
            xb = x[b]
            if hi == 1:
                xb = np.concatenate([xb[NQ:], xb[:NQ]], axis=0)
            in_maps.append(
                {
                    "x": np.ascontiguousarray(xb),
                    "gamma": gamma,
                    "beta": beta,
                    "Wq": Wq,
                    "Wk": Wk,
                    "Wv": Wv,
                    "Wo": Wo,
                    "nk": nk,
                    "nv": nv,
                    "qs": q_scale,
                    "ks": k_scale,
                }
            )

    res = run_bass_kernel_spmd(nc, in_maps, list(range(8)))

    full = np.empty((B, N, DIM), dtype=np.float32)
    for c in range(8):
        b, hi = divmod(c, 2)
        full[b, hi * NQ : (hi + 1) * NQ] = res.results[c]["out"]
    return full


# revision 17
# speedup vs baseline: 1.6579x; 1.2752x over previous
"""Trainium2 Bass kernel for nn_Attention (LayerNorm + L2-normalized-QK attention
with null-kv slot + output projection), SPMD across 8 NeuronCores.

Sharding: core c = (batch b = c//2, query-half hi = c%2). Each core computes the
full kv (2048 tokens) of its batch and attention outputs for its 1024-query
half. Softmax over kv is permutation invariant, so for hi=1 we feed x with the
two sequence halves swapped — every core then runs the identical SPMD program
with its queries in rows 0:1024. The final output is a pure concatenation of
the per-core results (no collectives, no host arithmetic).

Device-side layout choices:
  - S is computed transposed (S^T [kv, q]) so no softmax row-max pass is
    needed: q,k are L2-normalized so |8*q.k| <= 8 and exp() cannot overflow.
  - PV uses V' = [V | 1] (M=65) so the softmax denominator falls out of the
    same matmul chain, and the output lands directly in A^T layout for the
    output projection.
  - rsqrt is computed as exp(-0.5*ln(x)) so the only ACT table set used is
    natural_log_exp_and_others (no table thrashing with the softmax exp).
  - all matmuls in bf16 with fp32 PSUM accumulation.
"""

import numpy as np

B = 4
N = 2048
DIM = 1024
HEADS = 16
DH = 64
INNER = HEADS * DH
NQ = 1024  # queries per core
SCALE = 8.0
LN_EPS = 1e-5

_CACHE = {}


def _build_program(beta_zero: bool):
    from contextlib import ExitStack

    import concourse.bacc as bacc
    import concourse.bass as bass
    import concourse.tile as tile
    from concourse import mybir

    f32 = mybir.dt.float32
    bf16 = mybir.dt.bfloat16
    AF = mybir.ActivationFunctionType
    OP = mybir.AluOpType
    AX = mybir.AxisListType

    NT = N // 128          # 16 token tiles
    NTQ = NQ // 128        # 8 query token tiles
    NCD = DIM // 128       # 8 dim chunks
    HP = HEADS // 2        # 8 head pairs

    nc = bacc.Bacc("TRN2", target_bir_lowering=False, debug=False)

    x = nc.declare_dram_parameter("x", [N, DIM], f32, isOutput=False)
    gamma = nc.declare_dram_parameter("gamma", [DIM], f32, isOutput=False)
    beta = nc.declare_dram_parameter("beta", [DIM], f32, isOutput=False)
    Wq = nc.declare_dram_parameter("Wq", [DIM, INNER], f32, isOutput=False)
    Wk = nc.declare_dram_parameter("Wk", [DIM, INNER], f32, isOutput=False)
    Wv = nc.declare_dram_parameter("Wv", [DIM, INNER], f32, isOutput=False)
    Wo = nc.declare_dram_parameter("Wo", [INNER, DIM], f32, isOutput=False)
    nk = nc.declare_dram_parameter("nk", [HEADS, DH], f32, isOutput=False)
    nv = nc.declare_dram_parameter("nv", [HEADS, DH], f32, isOutput=False)
    qs = nc.declare_dram_parameter("qs", [DH], f32, isOutput=False)
    ks = nc.declare_dram_parameter("ks", [DH], f32, isOutput=False)
    out = nc.declare_dram_parameter("out", [NQ, DIM], f32, isOutput=True)

    # internal DRAM for transpose round-trips
    xn_d = nc.dram_tensor("xn_d", [N, DIM], bf16)
    kn_d = nc.dram_tensor("kn_d", [N, INNER], bf16)
    qn_d = nc.dram_tensor("qn_d", [NQ, INNER], bf16)
    nkn_d = nc.dram_tensor("nkn_d", [HEADS, DH], bf16)
    nvb_d = nc.dram_tensor("nvb_d", [HEADS, DH + 1], bf16)
    rcp_d = nc.dram_tensor("rcp_d", [HEADS, 2, DH, 8], f32)
    den_d = nc.dram_tensor("den_d", [HEADS, 2, DH, 8], f32)

    with tile.TileContext(nc) as tc, ExitStack() as ctx:
        singles = ctx.enter_context(tc.tile_pool(name="singles", bufs=1))
        big = ctx.enter_context(tc.tile_pool(name="big", bufs=1))

        # ---------------- persistent SBUF tensors ----------------
        xnT = big.tile([128, NCD, N], bf16, tag="xnT")       # xn^T  [dim, tok]
        kT = big.tile([128, HP, N], bf16, tag="kT")          # k^T   [2*64, kv] per pair
        qT = big.tile([128, HP, NQ], bf16, tag="qT")         # q^T
        vsb = big.tile([128, NT, HEADS, DH + 1], bf16, tag="v")   # V'=[V|1]
        AT = big.tile([128, NCD, NQ], bf16, tag="AT")        # A^T (attn out)

        # ---------------- constants ----------------
        gamma_b = singles.tile([128, DIM], f32)
        nc.gpsimd.dma_start(out=gamma_b, in_=gamma.ap().partition_broadcast(128))
        beta_b = singles.tile([128, DIM], f32)
        nc.gpsimd.dma_start(out=beta_b, in_=beta.ap().partition_broadcast(128))
        eps_t = singles.tile([128, 1], f32)
        nc.vector.memset(eps_t, LN_EPS)
        eps30 = singles.tile([128, 1], f32)
        nc.vector.memset(eps30, 1e-30)

        qs_b = singles.tile([128, DH], f32)
        nc.gpsimd.dma_start(out=qs_b, in_=qs.ap().partition_broadcast(128))
        ks_b = singles.tile([128, DH], f32)
        nc.gpsimd.dma_start(out=ks_b, in_=ks.ap().partition_broadcast(128))
        c64 = singles.tile([128, DH], f32)
        nc.vector.tensor_tensor(out=c64, in0=qs_b, in1=ks_b, op=OP.mult)
        c8 = singles.tile([128, 8, DH], f32)   # qs*ks tiled for 8 heads (one col half)
        for g in range(8):
            nc.vector.tensor_copy(out=c8[:, g, :], in_=c64)

        nc.vector.memset(vsb[:, :, :, DH : DH + 1], 1.0)  # ones column of V'

        # null-kv prep: nkn = l2norm(nk)*qs*ks (bf16), then transpose via DRAM
        nk_t = singles.tile([HEADS, DH], f32)
        nc.sync.dma_start(out=nk_t, in_=nk[:, :])
        nksq = singles.tile([HEADS, DH], f32)
        nc.vector.tensor_tensor(out=nksq, in0=nk_t, in1=nk_t, op=OP.mult)
        nks = singles.tile([HEADS, 1], f32)
        nc.vector.tensor_reduce(out=nks, in_=nksq, axis=AX.X, op=OP.add)
        nc.scalar.activation(out=nks, in_=nks, func=AF.Ln, bias=eps30[0:HEADS, :])
        nc.scalar.activation(out=nks, in_=nks, func=AF.Exp, scale=-0.5)
        nc.vector.tensor_scalar_min(out=nks, in0=nks, scalar1=1e12)
        nkn = singles.tile([HEADS, DH], f32)
        nc.vector.tensor_scalar_mul(out=nkn, in0=nk_t, scalar1=nks)
        nknb = singles.tile([HEADS, DH], bf16)
        nc.vector.tensor_tensor(out=nknb, in0=nkn, in1=c64[0:HEADS, :], op=OP.mult)
        nc.sync.dma_start(out=nkn_d[:, :], in_=nknb)
        nknT = singles.tile([DH, HEADS], bf16)
        nc.sync.dma_start(out=nknT, in_=nkn_d.ap().rearrange("h d -> d h"))
        # block-diagonal [128, 16]: col 2p rows 0:64 = head 2p, col 2p+1 rows 64:128 = head 2p+1
        nkn_bd = singles.tile([128, HEADS], bf16)
        nc.vector.memset(nkn_bd, 0.0)
        nc.sync.dma_start(out=nkn_bd[0:DH, 0:HEADS:2], in_=nknT[:, 0:HEADS:2])
        nc.sync.dma_start(out=nkn_bd[DH:128, 1:HEADS:2], in_=nknT[:, 1:HEADS:2])

        # null-v: nv_bd2 [2, HEADS, DH+1]; row parity selects head parity
        nv_t = singles.tile([HEADS, DH], f32)
        nc.sync.dma_start(out=nv_t, in_=nv[:, :])
        nvb = singles.tile([HEADS, DH + 1], bf16)
        nc.vector.tensor_copy(out=nvb[:, 0:DH], in_=nv_t)
        nc.vector.memset(nvb[:, DH : DH + 1], 1.0)
        nv_bd2 = singles.tile([2, HEADS, DH + 1], bf16)
        nc.vector.memset(nv_bd2, 0.0)
        nc.sync.dma_start(out=nvb_d[:, :], in_=nvb)
        nc.sync.dma_start(
            out=nv_bd2[0:1, 0:HEADS:2, :],
            in_=nvb_d.ap()[0:HEADS:2, :].partition_broadcast(1),
        )
        nc.sync.dma_start(
            out=nv_bd2[1:2, 1:HEADS:2, :],
            in_=nvb_d.ap()[1:HEADS:2, :].partition_broadcast(1),
        )



        # ---------------- phase 1: LayerNorm ----------------
        # beta_zero fast path: write xnu = (x - mu) * gamma (per-token rstd
        # scale commutes through the projections: q/k l2norms cancel it and
        # v-proj re-applies it as a per-partition scalar).
        mv_all = singles.tile([128, NT, 2], f32)
        rst_all = singles.tile([128, NT], f32)
        with (
            tc.tile_pool(name="px", bufs=3) as px,
            tc.tile_pool(name="pst", bufs=4) as pst,
            tc.tile_pool(name="pxc", bufs=2) as pxc,
            tc.tile_pool(name="pxn", bufs=2) as pxn,
        ):
            if beta_zero:
                for tt in range(NT):
                    r0 = tt * 128
                    xt = px.tile([128, DIM], f32)
                    nc.sync.dma_start(out=xt, in_=x[r0 : r0 + 128, :])
                    stats = pst.tile([128, 2, 6], f32, tag="stats")
                    nc.vector.bn_stats(out=stats[:, 0, :], in_=xt[:, 0:512])
                    nc.vector.bn_stats(out=stats[:, 1, :], in_=xt[:, 512:1024])
                    nc.vector.bn_aggr(out=mv_all[:, tt, :], in_=stats)
                    xc = pxc.tile([128, DIM], f32)
                    nc.vector.tensor_scalar_sub(
                        out=xc, in0=xt, scalar1=mv_all[:, tt, 0:1]
                    )
                    xnt = pxn.tile([128, DIM], bf16)
                    nc.vector.tensor_tensor(out=xnt, in0=xc, in1=gamma_b, op=OP.mult)
                    nc.sync.dma_start(out=xn_d[r0 : r0 + 128, :], in_=xnt)
                    if tt % 4 == 3:
                        g0 = (tt - 3) * 128
                        for c in range(NCD):
                            nc.sync.dma_start(
                                out=xnT[:, c, g0 : g0 + 512],
                                in_=xn_d[g0 : g0 + 512, c * 128 : (c + 1) * 128],
                                transpose=True,
                            )
                # batched rstd (only v-proj consumes it)
                nc.scalar.activation(
                    out=rst_all, in_=mv_all[:, :, 1], func=AF.Ln, bias=eps_t
                )
                nc.scalar.activation(
                    out=rst_all, in_=rst_all, func=AF.Exp, scale=-0.5
                )
            else:
                for tt in range(NT):
                    r0 = tt * 128
                    xt = px.tile([128, DIM], f32)
                    nc.sync.dma_start(out=xt, in_=x[r0 : r0 + 128, :])
                    stats = pst.tile([128, 2, 6], f32, tag="stats")
                    nc.vector.bn_stats(out=stats[:, 0, :], in_=xt[:, 0:512])
                    nc.vector.bn_stats(out=stats[:, 1, :], in_=xt[:, 512:1024])
                    nc.vector.bn_aggr(out=mv_all[:, tt, :], in_=stats)
                nc.scalar.activation(
                    out=rst_all, in_=mv_all[:, :, 1], func=AF.Ln, bias=eps_t
                )
                nc.scalar.activation(
                    out=rst_all, in_=rst_all, func=AF.Exp, scale=-0.5
                )
                for tt in range(NT):
                    r0 = tt * 128
                    xt = px.tile([128, DIM], f32)
                    nc.sync.dma_start(out=xt, in_=x[r0 : r0 + 128, :])
                    xc = pxc.tile([128, DIM], f32)
                    nc.vector.tensor_scalar(
                        out=xc, in0=xt, scalar1=mv_all[:, tt, 0:1],
                        scalar2=rst_all[:, tt : tt + 1],
                        op0=OP.subtract, op1=OP.mult,
                    )
                    nc.vector.tensor_tensor(out=xc, in0=xc, in1=gamma_b, op=OP.mult)
                    xnt = pxn.tile([128, DIM], bf16)
                    nc.vector.tensor_tensor(out=xnt, in0=xc, in1=beta_b, op=OP.add)
                    nc.sync.dma_start(out=xn_d[r0 : r0 + 128, :], in_=xnt)
                    if tt % 4 == 3:
                        g0 = (tt - 3) * 128
                        for c in range(NCD):
                            nc.sync.dma_start(
                                out=xnT[:, c, g0 : g0 + 512],
                                in_=xn_d[g0 : g0 + 512, c * 128 : (c + 1) * 128],
                                transpose=True,
                            )

        # ---------------- phase 3: projections ----------------
        def load_w_half(pw, pwst, W, half):
            wt = pw.tile([128, NCD, 512], bf16, tag="W")
            for c in range(NCD):
                ws = pwst.tile([128, 512], f32, tag="wstage")
                nc.sync.dma_start(
                    out=ws, in_=W[c * 128 : (c + 1) * 128, half * 512 : (half + 1) * 512]
                )
                nc.vector.tensor_copy(out=wt[:, c, :], in_=ws)
            return wt

        with (
            tc.tile_pool(name="pw", bufs=2) as pw,
            tc.tile_pool(name="pwst", bufs=3) as pwst,
            tc.tile_pool(name="ppj", bufs=3, space="PSUM") as ppj,
            tc.tile_pool(name="pnrm", bufs=2) as pnrm,
            tc.tile_pool(name="pout", bufs=1) as pout,
        ):
            # ---- k projection (all 2048 tokens), l2norm along d, no scale
            for half in range(2):
                wk = load_w_half(pw, pwst, Wk, half)
                s_all = pnrm.tile([128, NT, 8], f32, tag="sall")
                ksb = []
                for tt in range(NT):
                    r0 = tt * 128
                    kp = ppj.tile([128, 512], f32, tag="pj")
                    for c in range(NCD):
                        nc.tensor.matmul(
                            kp, lhsT=xnT[:, c, r0 : r0 + 128], rhs=wk[:, c, :],
                            start=(c == 0), stop=(c == NCD - 1),
                        )
                    sq = pnrm.tile([128, 512], f32, tag="sq")
                    nc.scalar.activation(out=sq, in_=kp, func=AF.Square)
                    nc.vector.tensor_reduce(
                        out=s_all[:, tt, :], in_=sq.rearrange("p (g d) -> p g d", g=8),
                        axis=AX.X, op=OP.add,
                    )
                    ks_t = pout.tile([128, 512], bf16, tag=f"ksb{tt}")
                    nc.vector.tensor_copy(out=ks_t, in_=kp)
                    ksb.append(ks_t)
                # batched rsqrt for the whole half (one Ln/Exp table pair)
                nc.scalar.activation(
                    out=s_all.rearrange("p a b -> p (a b)"),
                    in_=s_all.rearrange("p a b -> p (a b)"), func=AF.Ln, bias=eps30,
                )
                nc.scalar.activation(
                    out=s_all.rearrange("p a b -> p (a b)"),
                    in_=s_all.rearrange("p a b -> p (a b)"), func=AF.Exp, scale=-0.5,
                )
                nc.vector.tensor_scalar_min(
                    out=s_all.rearrange("p a b -> p (a b)"),
                    in0=s_all.rearrange("p a b -> p (a b)"), scalar1=1e12,
                )
                for tt in range(NT):
                    r0 = tt * 128
                    rex = pnrm.tile([128, 8, DH], f32, tag="rex")
                    nc.vector.tensor_copy(
                        out=rex, in_=s_all[:, tt, :].broadcast_to([128, 8, DH])
                    )
                    knf = pnrm.tile([128, 512], bf16, tag="knf")
                    nc.vector.tensor_tensor(out=knf, in0=ksb[tt], in1=rex, op=OP.mult)
                    nc.sync.dma_start(
                        out=kn_d[r0 : r0 + 128, half * 512 : (half + 1) * 512], in_=knf
                    )

            # ---- q projection (first 1024 tokens), l2norm, * (qs*ks)
            last_qnorm_exp = None
            for half in range(2):
                wq = load_w_half(pw, pwst, Wq, half)
                s_all = pnrm.tile([128, NTQ, 8], f32, tag="sallq")
                qsb = []
                for tt in range(NTQ):
                    r0 = tt * 128
                    qp = ppj.tile([128, 512], f32, tag="pj")
                    for c in range(NCD):
                        nc.tensor.matmul(
                            qp, lhsT=xnT[:, c, r0 : r0 + 128], rhs=wq[:, c, :],
                            start=(c == 0), stop=(c == NCD - 1),
                        )
                    sq = pnrm.tile([128, 512], f32, tag="sq")
                    nc.scalar.activation(out=sq, in_=qp, func=AF.Square)
                    nc.vector.tensor_reduce(
                        out=s_all[:, tt, :], in_=sq.rearrange("p (g d) -> p g d", g=8),
                        axis=AX.X, op=OP.add,
                    )
                    qs_t = pout.tile([128, 512], bf16, tag=f"qsb{tt}")
                    nc.vector.tensor_copy(out=qs_t, in_=qp)
                    qsb.append(qs_t)
                nc.scalar.activation(
                    out=s_all.rearrange("p a b -> p (a b)"),
                    in_=s_all.rearrange("p a b -> p (a b)"), func=AF.Ln, bias=eps30,
                )
                last_qnorm_exp = nc.scalar.activation(
                    out=s_all.rearrange("p a b -> p (a b)"),
                    in_=s_all.rearrange("p a b -> p (a b)"), func=AF.Exp, scale=-0.5,
                )
                nc.vector.tensor_scalar_min(
                    out=s_all.rearrange("p a b -> p (a b)"),
                    in0=s_all.rearrange("p a b -> p (a b)"), scalar1=1e12,
                )
                for tt in range(NTQ):
                    r0 = tt * 128
                    rex = pnrm.tile([128, 8, DH], f32, tag="rex")
                    nc.vector.tensor_copy(
                        out=rex, in_=s_all[:, tt, :].broadcast_to([128, 8, DH])
                    )
                    rc = pnrm.tile([128, 512], f32, tag="qn1")
                    nc.vector.tensor_tensor(
                        out=rc, in0=rex.rearrange("p g d -> p (g d)"),
                        in1=c8.rearrange("p g d -> p (g d)"), op=OP.mult,
                    )
                    qnf = pnrm.tile([128, 512], bf16, tag="knf")
                    nc.vector.tensor_tensor(out=qnf, in0=qsb[tt], in1=rc, op=OP.mult)
                    nc.sync.dma_start(
                        out=qn_d[r0 : r0 + 128, half * 512 : (half + 1) * 512], in_=qnf
                    )

            # ---- k^T / q^T via DRAM transpose
            for p in range(HP):
                nc.sync.dma_start(
                    out=kT[:, p, :], in_=kn_d[:, p * 128 : (p + 1) * 128], transpose=True
                )
                nc.sync.dma_start(
                    out=qT[:, p, :], in_=qn_d[:, p * 128 : (p + 1) * 128], transpose=True
                )

            # ---- v projection (all tokens) -> V' natural layout
            for half in range(2):
                wv = load_w_half(pw, pwst, Wv, half)
                for tt in range(NT):
                    r0 = tt * 128
                    vp = ppj.tile([128, 512], f32, tag="pj")
                    for c in range(NCD):
                        nc.tensor.matmul(
                            vp, lhsT=xnT[:, c, r0 : r0 + 128], rhs=wv[:, c, :],
                            start=(c == 0), stop=(c == NCD - 1),
                        )
                    if beta_zero:
                        nc.vector.tensor_scalar_mul(
                            out=vsb[:, tt, half * 8 : (half + 1) * 8, 0:DH],
                            in0=vp.rearrange("p (g d) -> p g d", g=8),
                            scalar1=rst_all[:, tt : tt + 1],
                        )
                    else:
                        nc.vector.tensor_copy(
                            out=vsb[:, tt, half * 8 : (half + 1) * 8, 0:DH],
                            in_=vp.rearrange("p (g d) -> p g d", g=8),
                        )

        # ---------------- phase 5: attention ----------------
        QB = NQ // 512  # 2 query blocks of 512
        with (
            tc.tile_pool(name="pstt", bufs=2, space="PSUM") as pstt,
            tc.tile_pool(name="pot", bufs=4, space="PSUM") as pot,
            tc.tile_pool(name="ppt", bufs=3) as ppt,
            tc.tile_pool(name="pptn", bufs=2) as pptn,
            tc.tile_pool(name="prec", bufs=2) as prec,
            tc.tile_pool(name="pbsc", bufs=2) as pbsc,
        ):
            first_attn_exp = None
            for hp in range(HP):
                hA, hB = 2 * hp, 2 * hp + 1
                for qb in range(QB):
                    q0 = qb * 512
                    # null scores for both heads: [2, 512] (in an st slot)
                    st_n = pstt.tile([128, 2, 512], f32, tag="st")
                    null_ps = st_n[0:2, 0, :]
                    nc.tensor.matmul(
                        null_ps, lhsT=nkn_bd[:, hA : hA + 2],
                        rhs=qT[:, hp, q0 : q0 + 512], start=True, stop=True,
                    )
                    pTn = pptn.tile([2, 512], bf16)
                    e = nc.scalar.activation(
                        out=pTn, in_=null_ps, func=AF.Exp, scale=SCALE
                    )
                    if first_attn_exp is None:
                        first_attn_exp = e
                        if last_qnorm_exp is not None:
                            tile.add_dep_helper(
                                first_attn_exp.ins, last_qnorm_exp.ins,
                                reason="keep ACT exp phase after all Ln/Exp batches",
                            )

                    otA = pot.tile([DH + 1, 512], f32, tag="ot")
                    otB = pot.tile([DH + 1, 512], f32, tag="ot")

                    for c in range(16):
                        st = pstt.tile([128, 2, 512], f32, tag="st")
                        for si, (h, rh) in enumerate(((hA, 0), (hB, 1))):
                            nc.tensor.matmul(
                                st[:, si, :],
                                lhsT=kT[rh * DH : (rh + 1) * DH, hp, c * 128 : (c + 1) * 128],
                                rhs=qT[rh * DH : (rh + 1) * DH, hp, q0 : q0 + 512],
                                start=True, stop=True,
                                tile_position=(rh * DH, 0),
                            )
                        pt = ppt.tile([128, 2, 512], bf16)
                        nc.scalar.activation(out=pt, in_=st, func=AF.Exp, scale=SCALE)
                        for si, (ot, h) in enumerate(((otA, hA), (otB, hB))):
                            nc.tensor.matmul(
                                ot, lhsT=vsb[:, c, h, :], rhs=pt[:, si, :],
                                start=(c == 0), stop=False,
                            )
                    # null PV (finishes accumulation)
                    nc.tensor.matmul(
                        otA, lhsT=nv_bd2[:, hA, :], rhs=pTn, start=False, stop=True
                    )
                    nc.tensor.matmul(
                        otB, lhsT=nv_bd2[:, hB, :], rhs=pTn, start=False, stop=True
                    )
                    # divide by denominator (row DH of ot) and write A^T
                    for h, ot in ((hA, otA), (hB, otB)):
                        den_s = prec.tile([1, 512], f32, tag="dens")
                        nc.vector.tensor_copy(out=den_s, in_=ot[DH : DH + 1, :])
                        nc.sync.dma_start(
                            out=den_d[h, qb].rearrange("a b -> (a b)").partition_broadcast(1),
                            in_=den_s,
                        )
                        dd = prec.tile([DH, 8], f32, tag="dd")
                        nc.sync.dma_start(out=dd, in_=den_d[h, qb])
                        rr = prec.tile([DH, 8], f32, tag="rr")
                        nc.vector.reciprocal(rr, dd)
                        nc.sync.dma_start(out=rcp_d[h, qb], in_=rr)
                        rcs = pbsc.tile([DH, 512], f32, tag="bcs")
                        nc.sync.dma_start(
                            out=rcs,
                            in_=rcp_d[h, qb].rearrange("a b -> (a b)").partition_broadcast(DH),
                        )
                        po = (h % 2) * DH
                        nc.vector.tensor_tensor(
                            out=AT[po : po + DH, h // 2, q0 : q0 + 512],
                            in0=ot[0:DH, :], in1=rcs, op=OP.mult,
                        )

        # ---------------- phase 6: output projection ----------------
        with (
            tc.tile_pool(name="pw2", bufs=2) as pw2,
            tc.tile_pool(name="pwst2", bufs=3) as pwst2,
            tc.tile_pool(name="ppj2", bufs=3, space="PSUM") as ppj2,
            tc.tile_pool(name="pob", bufs=3) as pob,
        ):
            for half in range(2):
                wo = load_w_half(pw2, pwst2, Wo, half)
                for tt in range(NTQ):
                    r0 = tt * 128
                    op_ = ppj2.tile([128, 512], f32)
                    for c in range(NCD):
                        nc.tensor.matmul(
                            op_, lhsT=AT[:, c, r0 : r0 + 128], rhs=wo[:, c, :],
                            start=(c == 0), stop=(c == NCD - 1),
                        )
                    ob = pob.tile([128, 512], f32)
                    nc.vector.tensor_copy(out=ob, in_=op_)
                    nc.sync.dma_start(
                        out=out[r0 : r0 + 128, half * 512 : (half + 1) * 512], in_=ob
                    )

    nc.compile()
    return nc


def _get_program(beta_zero: bool = True):
    key = ("nc", beta_zero)
    if key not in _CACHE:
        _CACHE[key] = _build_program(beta_zero)
    return _CACHE[key]


def kernel(**inputs) -> np.ndarray:
    from concourse.bass_utils import run_bass_kernel_spmd

    x = np.asarray(inputs["x"], dtype=np.float32)
    gamma = np.asarray(inputs["gamma"], dtype=np.float32)
    beta = np.asarray(inputs["beta"], dtype=np.float32)
    nc = _get_program(beta_zero=not np.any(beta))
    null_kv = np.asarray(inputs["null_kv"], dtype=np.float32)
    Wq = np.ascontiguousarray(np.asarray(inputs["Wq"], dtype=np.float32))
    Wkv = np.asarray(inputs["Wkv"], dtype=np.float32)
    q_scale = np.asarray(inputs["q_scale"], dtype=np.float32)
    k_scale = np.asarray(inputs["k_scale"], dtype=np.float32)
    Wo = np.ascontiguousarray(np.asarray(inputs["Wo"], dtype=np.float32))

    Wk = np.ascontiguousarray(Wkv[:, :INNER])
    Wv = np.ascontiguousarray(Wkv[:, INNER:])
    nk = np.ascontiguousarray(null_kv[0, :, 0, :])
    nv = np.ascontiguousarray(null_kv[1, :, 0, :])

    in_maps = []
    for b in range(B):
        for hi in range(2):
            xb = x[b]
            if hi == 1:
                xb = np.concatenate([xb[NQ:], xb[:NQ]], axis=0)
            in_maps.append(
                {
                    "x": np.ascontiguousarray(xb),
                    "gamma": gamma,
                    "beta": beta,
                    "Wq": Wq,
                    "Wk": Wk,
                    "Wv": Wv,
                    "Wo": Wo,
                    "nk": nk,
                    "nv": nv,
                    "qs": q_scale,
                    "ks": k_scale,
                }
            )

    res = run_bass_kernel_spmd(nc, in_maps, list(range(8)))

    full = np.empty((B, N, DIM), dtype=np.float32)
    for c in range(8):
        b, hi = divmod(c, 2)
        full[b, hi * NQ : (hi + 1) * NQ] = res.results[c]["out"]
    return full
